# revision 1
# baseline (speedup 1.0000x reference)
"""Trainium2 Bass kernel: causal attention (QKV projection + causal softmax + AV).

Problem: x[4, 4096, 768] fp32, per-head projections to d=64, full causal
attention per batch, output [4, 4096, 64] fp32.

Sharding: 8 cores = 4 batches x 2 parity groups. Core (b, j) computes the
output rows of batch b whose 128-row block index i satisfies i % 2 == j.
One uniform SPMD program: for j=0 cores the host shifts x down by one
128-row block (prepending zeros), which makes the causal structure of both
parities identical in device coordinates (device q-blocks are always the odd
blocks 1,3,...,31; k-slot g holds true block g-1 for j=0 and g for j=1; the
dead slot 0 of j=0 is zeroed post-exp with a per-core 0/1 scale).

Device pipeline per core (all matmuls bf16, fp32 PSUM accumulation):
  P1 (per 512-row seq chunk): one 3D-output DMA-transpose yields x^T for the
     chunk; two matmul passes with stationary [wq|wq] and [wk|wv] produce
     Q^T (own q-blocks, both partition halves), K^T (low half, SWDGE-
     duplicated to the high half) and V^T (DMA-transposed into V' = [V | 1]).
  P2 (per 512-col q chunk): for consecutive k-slot pairs, two concurrent
     row-tiled matmuls K^T_g.T @ Q^T produce S^T; exp on ACT (scale 1/8,
     causal diagonal masked by a bf16 upper-tri mask, merged across the
     pair's two PSUM banks); AV accumulates V'.T @ P^T into a [65, 512]
     PSUM tile whose row 64 is the softmax denominator (ones column of V').
     The unnormalized [65, 512] tiles go to DRAM; the host divides and
     transposes.
"""

import numpy as np
import ml_dtypes
from contextlib import ExitStack

import concourse.bass as bass
import concourse.mybir as mybir
import concourse.tile as tile
from concourse import bacc
from concourse.bass_utils import run_bass_kernel_spmd

F32 = mybir.dt.float32
BF16 = mybir.dt.bfloat16

SEQ = 4096
DIN = 768
DOUT = 64
NCC = DIN // 128          # 6 contraction chunks
NSC = SEQ // 512          # 8 seq chunks (projection granularity)
NBLK = SEQ // 128         # 32 k-slots
NQC = 4                   # q chunks of 512 local columns (2048 own q rows)
SCALE = 1.0 / 8.0
EXPF = mybir.ActivationFunctionType.Exp

_CACHED_NC = None


def build_nc(dump=False, repeats=1):
    nc = bacc.Bacc("TRN2", target_bir_lowering=False, debug=False)

    x = nc.dram_tensor("x", [SEQ, DIN], BF16, kind="ExternalInput")
    wqq = nc.dram_tensor("wqq", [DIN, 128], BF16, kind="ExternalInput")  # [wq|wq]
    wkv = nc.dram_tensor("wkv", [DIN, 128], BF16, kind="ExternalInput")  # [wv|wk]
    bqq = nc.dram_tensor("bqq", [128, 1], F32, kind="ExternalInput")     # [bq;bq]
    bkv = nc.dram_tensor("bkv", [128, 1], F32, kind="ExternalInput")     # [bk;bv]
    pads = nc.dram_tensor("pads", [128, 1], F32, kind="ExternalInput")   # 1 / 0
    maska = nc.dram_tensor("maska", [128, 512], BF16, kind="ExternalInput")
    idnb = nc.dram_tensor("idnb", [64, 64], BF16, kind="ExternalInput")
    o = nc.dram_tensor("o", [NQC, 65, 512], F32, kind="ExternalOutput")
    if dump:
        okt = nc.dram_tensor("okt", [64, NBLK * 128], BF16, kind="ExternalOutput")
        oqt = nc.dram_tensor("oqt", [128, 16 * 128], BF16, kind="ExternalOutput")
        ovs = nc.dram_tensor("ovs", [128, NBLK * 65], BF16, kind="ExternalOutput")
        opt = nc.dram_tensor("opt", [128, 1024], BF16, kind="ExternalOutput")

    with tile.TileContext(nc) as tc, ExitStack() as ctx:
        cpool = ctx.enter_context(tc.tile_pool(name="const", bufs=1))
        vtp = ctx.enter_context(tc.tile_pool(name="vt", bufs=2))
        ptp = ctx.enter_context(tc.tile_pool(name="pt", bufs=3))
        ocp = ctx.enter_context(tc.tile_pool(name="oc", bufs=2))
        psproj = ctx.enter_context(tc.tile_pool(name="psproj", bufs=2, space="PSUM"))
        psst = ctx.enter_context(tc.tile_pool(name="psst", bufs=2, space="PSUM"))
        psav = ctx.enter_context(tc.tile_pool(name="psav", bufs=2, space="PSUM"))

        wqq_sb = cpool.tile([128, NCC * 128], BF16)
        wkv_sb = cpool.tile([128, NCC * 128], BF16)
        bqq_sb = cpool.tile([128, 1], F32)
        bkv_sb = cpool.tile([128, 1], F32)
        pads_sb = cpool.tile([128, 1], F32)
        mask_sb = cpool.tile([128, 512], BF16)
        idn_sb = cpool.tile([64, 64], BF16)
        kt2 = cpool.tile([128, NBLK * 128], BF16)   # K^T, both partition halves
        xtf = cpool.tile([128, NSC * NCC * 512], BF16)  # x^T, whole sequence
        qt = cpool.tile([128, 16 * 128], BF16)      # Q^T own blocks, both halves
        vs = cpool.tile([128, NBLK * 65], BF16)     # V' = [V | 1] per k-slot

        # weights laid out [c-in-chunk partition, (chunk, out_col) free]
        nc.sync.dma_start(
            wqq_sb[:].rearrange("p (cc m) -> p cc m", cc=NCC),
            wqq.rearrange("(cc p) m -> p cc m", p=128),
        )
        nc.sync.dma_start(
            wkv_sb[:].rearrange("p (cc m) -> p cc m", cc=NCC),
            wkv.rearrange("(cc p) m -> p cc m", p=128),
        )
        nc.sync.dma_start(bqq_sb[:], bqq[:, :])
        nc.sync.dma_start(bkv_sb[:], bkv[:, :])
        nc.sync.dma_start(pads_sb[:], pads[:, :])
        nc.sync.dma_start(mask_sb[:], maska[:, :])
        nc.sync.dma_start(idn_sb[:], idnb[:, :])
        # ones column of V'
        nc.vector.memset(
            vs[:].rearrange("p (g e) -> p g e", g=NBLK)[:, :, 64:65], 1.0
        )

        def xts(sc, cc):
            base = sc * NCC * 512 + cc * 512
            return xtf[:, base:base + 512]

        def trans_chunk(sc):
            """DMA-transpose x rows [sc*512, (sc+1)*512) into resident x^T."""
            nc.sync.dma_start_transpose(
                xtf[:, sc * NCC * 512:(sc + 1) * NCC * 512]
                .rearrange("p (cc s) -> p cc s", cc=NCC),
                x[sc * 512:(sc + 1) * 512, :],
            )

        def passA_chunk(sc):
            """Q^T for own (odd) q-blocks of this chunk, [wq|wq] stationary."""
            qp = psproj.tile([128, 256], F32, tag="proj")
            for cc in range(NCC):
                rhs = (
                    xts(sc, cc)
                    .rearrange("p (a b s) -> p a b s", a=2, b=2)[:, :, 1, :]
                )
                nc.tensor.matmul(
                    qp[:], wqq_sb[:, cc * 128:(cc + 1) * 128], rhs,
                    start=(cc == 0), stop=(cc == NCC - 1),
                )
            nc.vector.tensor_scalar_add(
                qt[:, sc * 256:(sc + 1) * 256], qp[:], bqq_sb[:]
            )

        def passB_chunk(sc):
            """K^T (rows 64-127) and V^T (rows 0-63), [wv|wk] stationary."""
            kp = psproj.tile([128, 512], F32, tag="proj")
            for cc in range(NCC):
                nc.tensor.matmul(
                    kp[:], wkv_sb[:, cc * 128:(cc + 1) * 128],
                    xts(sc, cc),
                    start=(cc == 0), stop=(cc == NCC - 1),
                )
            nc.vector.tensor_scalar_add(
                kt2[64:128, sc * 512:(sc + 1) * 512], kp[64:128, :], bkv_sb[64:128, :]
            )
            hi = kt2[64:128, sc * 512:(sc + 1) * 512].rearrange(
                "p (a b s) -> p a b s", a=2, b=2)[:, :, 0, :]
            lo = kt2[0:64, sc * 512:(sc + 1) * 512].rearrange(
                "p (a b s) -> p a b s", a=2, b=2)[:, :, 0, :]
            nc.gpsimd.dma_start(lo, hi)
            vt = vtp.tile([128, 512], BF16)
            nc.vector.tensor_scalar_add(
                vt[0:64, :], kp[0:64, :], bkv_sb[0:64, :]
            )
            # V' blocks via PE transpose (DMA-transpose is only HW-exact for
            # the whole-row DRAM-sourced x case)
            vp = psproj.tile([128, 256], BF16, tag="proj")
            for t in range(4):
                nc.tensor.transpose(
                    vp[:, t * 64:(t + 1) * 64],
                    vt[0:64, t * 128:(t + 1) * 128],
                    idn_sb[:],
                )
            nc.vector.tensor_copy(
                vs[:].rearrange("p (g e) -> p g e", g=NBLK)[
                    :, sc * 4:(sc + 1) * 4, 0:64
                ],
                vp[:].rearrange("p (g e) -> p g e", g=4),
            )

        parts = {}

        def attn_seg(c, p_lo, p_hi, final):
            """Attention pairs [p_lo, p_hi) for local q cols [c*512, (c+1)*512)."""
            npairs = 4 * c + 4           # k-slots 0..8c+7 in consecutive pairs
            av = psav.tile([65, 512], F32, tag="av")
            first_av = [True]

            def slot_geom(g):
                s = g - (8 * c + 1)
                if s < 1:
                    return 0, 512
                off = 128 * ((s + 1) // 2)
                return off, 512 - off

            for p in range(p_lo, p_hi):
                g0, g1 = 2 * p, 2 * p + 1
                off0, w0 = slot_geom(g0)
                off1, w1 = slot_geom(g1)
                st = psst.tile([128, 1024], F32, tag="st")
                nc.tensor.matmul(
                    st[:, 0:w0], kt2[0:64, g0 * 128:(g0 + 1) * 128],
                    qt[0:64, c * 512 + off0: c * 512 + off0 + w0],
                    start=True, stop=True, tile_position=(0, 0),
                )
                nc.tensor.matmul(
                    st[:, 512:512 + w1], kt2[64:128, g1 * 128:(g1 + 1) * 128],
                    qt[64:128, c * 512 + off1: c * 512 + off1 + w1],
                    start=True, stop=True, tile_position=(64, 0),
                )
                pt = ptp.tile([128, 1024], BF16)
                if w0 < 512:
                    # exact-width exps: skip the gap instead of memset+merge
                    nc.scalar.activation(pt[:, 0:w0], st[:, 0:w0],
                                         EXPF, bias=0.0, scale=SCALE)
                    nc.scalar.activation(pt[:, 512:512 + w1], st[:, 512:512 + w1],
                                         EXPF, bias=0.0, scale=SCALE)
                else:
                    nc.scalar.activation(pt[:, 0:512 + w1], st[:, 0:512 + w1],
                                         EXPF, bias=0.0, scale=SCALE)
                if p == 0:
                    # kill the j=0 dead slot 0 (pads = 0 there, 1 for j=1)
                    nc.vector.tensor_scalar_mul(
                        pt[:, 0:512], pt[:, 0:512], pads_sb[:]
                    )
                if p >= npairs - 4:
                    # odd member of the last four pairs is causal-diagonal
                    nc.vector.tensor_mul(
                        pt[:, 512:512 + w1], pt[:, 512:512 + w1], mask_sb[:, 0:w1]
                    )
                if dump and c == 0 and p == 0:
                    nc.sync.dma_start(opt[:, :], pt[:])
                nc.tensor.matmul(
                    av[:, off0:off0 + w0], vs[:, g0 * 65:(g0 + 1) * 65],
                    pt[:, 0:w0],
                    start=first_av[0], stop=False,
                )
                first_av[0] = False
                nc.tensor.matmul(
                    av[:, off1:off1 + w1], vs[:, g1 * 65:(g1 + 1) * 65],
                    pt[:, 512:512 + w1],
                    start=False, stop=(p == p_hi - 1),
                )
            if final:
                oc = ocp.tile([65, 512], F32)
                if c in parts:
                    nc.vector.tensor_add(oc[:], av[:], parts.pop(c)[:])
                else:
                    nc.vector.tensor_copy(oc[:], av[:])
                nc.gpsimd.dma_start(o[c, :, :], oc[:])
            else:
                part = ocp.tile([65, 512], F32, tag="part")
                nc.vector.tensor_copy(part[:], av[:])
                parts[c] = part

        # chunk 3's first attention half only needs k-slots 0-15 plus its own
        # Q columns (sc6/7): with x^T fully resident, project that Q early so
        # the exp load isn't all at the tail
        for _rep in range(repeats):
            for sc in range(NSC):
                trans_chunk(sc)
            passA_chunk(0)
            passB_chunk(0)
            passA_chunk(1)
            passB_chunk(1)
            attn_seg(0, 0, 4, True)
            passA_chunk(2)
            passB_chunk(2)
            passA_chunk(3)
            passB_chunk(3)
            attn_seg(1, 0, 8, True)
            passA_chunk(6)
            passA_chunk(7)
            attn_seg(3, 0, 8, False)
            passA_chunk(4)
            passB_chunk(4)
            passA_chunk(5)
            passB_chunk(5)
            attn_seg(2, 0, 12, True)
            passB_chunk(6)
            passB_chunk(7)
            attn_seg(3, 8, 16, True)
        if dump:
            nc.sync.dma_start(okt[:, :], kt2[64:128, :])
            nc.sync.dma_start(oqt[:, :], qt[:])
            nc.sync.dma_start(ovs[:, :], vs[:])

    nc.compile()
    return nc


def _get_nc():
    global _CACHED_NC
    if _CACHED_NC is None:
        _CACHED_NC = build_nc()
    return _CACHED_NC


def _host_inputs(x, wq, bq, wk, bk, wv, bv):
    bf = ml_dtypes.bfloat16
    wqq = np.concatenate([wq, wq], axis=1).astype(bf)
    wkv = np.concatenate([wv, wk], axis=1).astype(bf)
    bqq = np.concatenate([bq, bq])[:, None].astype(np.float32)
    bkv = np.concatenate([bv, bk])[:, None].astype(np.float32)
    tri = np.triu(np.ones((128, 128), np.float32))
    maska = np.concatenate([tri, np.ones((128, 384), np.float32)], axis=1).astype(bf)
    idnb = np.eye(64, dtype=np.float32).astype(bf)
    xbf = np.ascontiguousarray(x).astype(bf)

    in_maps = []
    for core in range(8):
        b, j = core // 2, core % 2
        if j == 0:
            xdev = np.concatenate(
                [np.zeros((128, DIN), bf), xbf[b][: SEQ - 128]], axis=0
            )
            ps = np.zeros((128, 1), np.float32)
        else:
            xdev = xbf[b]
            ps = np.ones((128, 1), np.float32)
        in_maps.append({
            "x": np.ascontiguousarray(xdev),
            "wqq": wqq, "wkv": wkv, "bqq": bqq, "bkv": bkv,
            "pads": ps, "maska": maska, "idnb": idnb,
        })
    return in_maps


def _assemble(results):
    out = np.empty((4, SEQ, DOUT), np.float32)
    for core in range(8):
        b, j = core // 2, core % 2
        od = results[core]["o"]  # [NQC, 65, 512]
        for c in range(NQC):
            num = od[c, 0:64, :].astype(np.float64)
            den = od[c, 64, :].astype(np.float64)
            oc = (num / den).T.astype(np.float32)  # [512, 64]
            for t in range(4):
                r0 = (8 * c + 2 * t + j) * 128
                out[b, r0:r0 + 128] = oc[t * 128:(t + 1) * 128]
    return out


def kernel(x, wq, bq, wk, bk, wv, bv):
    x = np.asarray(x, dtype=np.float32)
    args = [np.asarray(a, dtype=np.float32) for a in (wq, bq, wk, bk, wv, bv)]
    nc = _get_nc()
    in_maps = _host_inputs(x, *args)
    br = run_bass_kernel_spmd(nc, in_maps, core_ids=list(range(8)))
    return _assemble(br.results)



# revision 13
# speedup vs baseline: 1.3307x; 1.3307x over previous
"""Trainium2 Bass kernel: causal attention (QKV projection + causal softmax + AV).

Problem: x[4, 4096, 768] fp32, per-head projections to d=64, full causal
attention per batch, output [4, 4096, 64] fp32.

Sharding: 8 cores = 4 batches x 2 parity groups. Core (b, j) computes the
output rows of batch b whose 128-row block index i satisfies i % 2 == j.
One uniform SPMD program: for j=0 cores the host shifts x down by one
128-row block (prepending zeros), which makes the causal structure of both
parities identical in device coordinates (device q-blocks are always the odd
blocks 1,3,...,31; k-slot g holds true block g-1 for j=0 and g for j=1; the
dead slot 0 of j=0 is killed with a per-core -30000 exp bias).

Math shortcuts: bk is dropped (adds a per-row constant to scores ->
softmax-invariant); bv is added on the host after normalization
(sum of softmax weights is 1). Only bq is applied on device.

Device pipeline per core (all matmuls bf16, fp32 PSUM accumulation):
  proj: DMA-transposed x^T chunks (issued across 4 DGE queues) feed
     passB ([wv|wk] stationary -> V rows 0:64 / K rows 64:128 of PSUM;
     K copied to SBUF hi-half by Pool, V PE-transposed into V' = [V | 1])
     and passA (wq stationary -> Q^T in partitions 64:128 + bq via DVE).
  attn (per 512-col q chunk): per k-slot pair, two matmuls K^T_g.T @ Q^T
     into a [128, 1024] PSUM tile; exp on ACT (scale 1/8, per-core dead-slot
     bias, bf16 out); causal-diagonal mask mul on DVE; AV accumulates
     V'.T @ P^T into a [65, 512] PSUM tile whose row 64 is the softmax
     denominator. Scores run one pair ahead of AV and projection matmuls
     fill remaining PE slack so the PE never idles (keeps max p-state).
The unnormalized [65, 512] tiles go to DRAM; the host divides, adds bv,
and transposes.
"""

import numpy as np
import ml_dtypes
from contextlib import ExitStack

import concourse.bass as bass
import concourse.mybir as mybir
import concourse.tile as tile
from concourse import bacc
from concourse.bass_utils import run_bass_kernel_spmd

F32 = mybir.dt.float32
BF16 = mybir.dt.bfloat16

SEQ = 4096
DIN = 768
DOUT = 64
NCC = DIN // 128          # 6 contraction chunks
NSC = SEQ // 512          # 8 seq chunks (projection granularity)
NBLK = SEQ // 128         # 32 k-slots
NQC = 4                   # q chunks of 512 local columns (2048 own q rows)
SCALE = 1.0 / 8.0
EXPF = mybir.ActivationFunctionType.Exp

_CACHED_NC = None


def build_nc(repeats=1):
    nc = bacc.Bacc("TRN2", target_bir_lowering=False, debug=False)

    x = nc.dram_tensor("x", [SEQ, DIN], BF16, kind="ExternalInput")
    # weights pre-transposed on host: rows = [wv|wk|wq] out-cols (192)
    wT = nc.dram_tensor("wT", [192, DIN], BF16, kind="ExternalInput")
    # misc consts pre-transposed: [mask(512) | idn(64) | bq(1) | dead(1) | pad]
    mT = nc.dram_tensor("mT", [592, 128], BF16, kind="ExternalInput")
    o = nc.dram_tensor("o", [NQC, 65, 512], F32, kind="ExternalOutput")

    with tile.TileContext(nc) as tc, ExitStack() as ctx:
        cpool = ctx.enter_context(tc.tile_pool(name="const", bufs=1))
        ptp = ctx.enter_context(tc.tile_pool(name="pt", bufs=3))
        ocp = ctx.enter_context(tc.tile_pool(name="oc", bufs=2))
        psproj = ctx.enter_context(tc.tile_pool(name="psproj", bufs=2, space="PSUM"))
        psst = ctx.enter_context(tc.tile_pool(name="psst", bufs=2, space="PSUM"))
        psav = ctx.enter_context(tc.tile_pool(name="psav", bufs=2, space="PSUM"))

        wsb = cpool.tile([128, NCC * 192], BF16)    # [wv|wk|wq] per cc chunk
        mc = cpool.tile([128, 592], BF16)           # mask | idn | bq | dead
        kvt = cpool.tile([128, NSC * 512], BF16)    # K^T rows 64:128, V^T rows 0:64
        xtf = cpool.tile([128, NSC * NCC * 512], BF16)  # x^T, whole sequence
        qt = cpool.tile([128, 16 * 128], BF16)      # Q^T own blocks, rows 64:128
        vs = cpool.tile([128, NBLK * 65], BF16)     # V' = [V | 1] per k-slot

        # ---- DMA issue plan: spread across 4 DGE queues so transfers
        # overlap and the first x^T chunk lands ~2us in.
        def trans(sc, eng, half=None):
            """DMA-transpose x rows of chunk sc (or half of it) into x^T."""
            if half is None:
                r0, r1, c0 = sc * 512, (sc + 1) * 512, 0
            else:
                r0 = sc * 512 + half * 256
                r1 = r0 + 256
                c0 = half * 256
            cols = r1 - r0
            eng.dma_start_transpose(
                xtf[:, sc * NCC * 512:(sc + 1) * NCC * 512]
                .rearrange("p (cc s) -> p cc s", cc=NCC)[:, :, c0:c0 + cols],
                x[r0:r1, :],
            )

        # All loads are DmaTransposeAnt on the single sync (SP) queue: the
        # tile framework chains every DMA issue to the previous DMA's
        # completion EXCEPT consecutive same-kind same-queue DMAs, which
        # pipeline back-to-back on the (serial) DMA engine device.
        nc.sync.dma_start_transpose(
            wsb[:].rearrange("p (cc m) -> p cc m", cc=NCC), wT[:, :]
        )
        trans(0, nc.sync, half=0)
        trans(0, nc.sync, half=1)
        nc.sync.dma_start_transpose(mc[:], mT[:, :])
        for _sc in range(1, NSC):
            trans(_sc, nc.sync)
        # f32 copies of bq / dead-slot bias (tensor_scalar needs f32 scalars)
        bqf = cpool.tile([128, 2], F32)
        nc.gpsimd.tensor_copy(bqf[:], mc[:, 576:578])
        # ones column of V'
        nc.vector.memset(
            vs[:].rearrange("p (g e) -> p g e", g=NBLK)[:, :, 64:65], 1.0
        )

        def xts(sc, cc):
            base = sc * NCC * 512 + cc * 512
            return xtf[:, base:base + 512]

        # ---- projection emitters; each returns a list of closures, one
        # per PE instruction (posts ride on the closure that needs them).
        proj_state = {}

        def passB_units(sc, split=False):
            """K^T rows 64:128 and V^T rows 0:64, [wv|wk] stationary."""
            units = []

            def mk(cc, c0, cols, start, stop):
                def f():
                    if cc == 0 and c0 == 0:
                        proj_state[("kp", sc)] = psproj.tile(
                            [128, 512], F32, tag="proj", name="kp")
                    kp = proj_state[("kp", sc)]
                    nc.tensor.matmul(
                        kp[:, c0:c0 + cols],
                        wsb[:, cc * 192:cc * 192 + 128],
                        xts(sc, cc)[:, c0:c0 + cols],
                        start=start, stop=stop,
                    )
                    if stop and c0 + cols == 512:
                        kp = proj_state.pop(("kp", sc))
                        # one copy: K rows 64:128 (no bias; bk is softmax-
                        # invariant) and V rows 0:64 land where each is used
                        nc.vector.tensor_copy(
                            kvt[:, sc * 512:(sc + 1) * 512], kp[:]
                        )
                return f

            if split:
                for c0 in (0, 256):
                    for cc in range(NCC):
                        units.append((("B", sc), mk(cc, c0, 256, cc == 0, cc == NCC - 1)))
            else:
                for cc in range(NCC):
                    units.append((("B", sc), mk(cc, 0, 512, cc == 0, cc == NCC - 1)))
            return units

        def vtrans_units(sc):
            """V' blocks via PE transpose of vt, then one DVE copy to vs."""
            units = []

            def mk(t):
                def f():
                    if t == 0:
                        proj_state[("vp", sc)] = psproj.tile(
                            [128, 256], BF16, tag="proj", name="vp")
                    vp = proj_state[("vp", sc)]
                    nc.tensor.transpose(
                        vp[:, t * 64:(t + 1) * 64],
                        kvt[0:64, sc * 512 + t * 128: sc * 512 + (t + 1) * 128],
                        mc[0:64, 512:576],
                    )
                    if t == 3:
                        vp = proj_state.pop(("vp", sc))
                        nc.vector.tensor_copy(
                            vs[:].rearrange("p (g e) -> p g e", g=NBLK)[
                                :, sc * 4:(sc + 1) * 4, 0:64
                            ],
                            vp[:].rearrange("p (g e) -> p g e", g=4),
                        )
                return f

            for t in range(4):
                units.append((("Vt", sc), mk(t)))
            return units

        def passA_units(sc):
            """Q^T for own (odd) q-blocks of this chunk, into rows 64:128."""
            units = []

            def mk(cc):
                def f():
                    if cc == 0:
                        proj_state[("qp", sc)] = psproj.tile(
                            [128, 256], F32, tag="proj", name="qp")
                    qp = proj_state[("qp", sc)]
                    rhs = (
                        xts(sc, cc)
                        .rearrange("p (a b s) -> p a b s", a=2, b=2)[:, :, 1, :]
                    )
                    nc.tensor.matmul(
                        qp[64:128, :], wsb[:, cc * 192 + 128:cc * 192 + 192], rhs,
                        start=(cc == 0), stop=(cc == NCC - 1),
                    )
                    if cc == NCC - 1:
                        qp = proj_state.pop(("qp", sc))
                        nc.vector.tensor_scalar_add(
                            qt[64:128, sc * 256:(sc + 1) * 256],
                            qp[64:128, :], bqf[64:128, 0:1],
                        )
                return f

            for cc in range(NCC):
                units.append((("A", sc), mk(cc)))
            return units

        # ---- attention ----
        def slot_geom(c, g):
            s = g - (8 * c + 1)
            if s < 1:
                return 0, 512
            off = 128 * ((s + 1) // 2)
            return off, 512 - off

        def attn_seg(c, fillers, per_pair, late=None):
            """Seg c: pairs of k-slots, scores one pair ahead of AV.

            fillers: ordered [(key, fn)] of projection units interleaved for
            PE occupancy; units a pair depends on are force-drained first.
            """
            npairs = 4 * c + 4
            av = psav.tile([65, 512], F32, tag="av", name="av")
            pending = []  # [(p, pt, geom)] awaiting AV

            def ensure(keys):
                while any(k in keys for k, _ in fillers):
                    _, fn = fillers.pop(0)
                    fn()

            def emit_scores(p):
                g0, g1 = 2 * p, 2 * p + 1
                off0, w0 = slot_geom(c, g0)
                off1, w1 = slot_geom(c, g1)
                st = psst.tile([128, 1024], F32, tag="st", name="st")
                nc.tensor.matmul(
                    st[:, 0:w0], kvt[64:128, g0 * 128:(g0 + 1) * 128],
                    qt[64:128, c * 512 + off0: c * 512 + off0 + w0],
                    start=True, stop=True,
                )
                nc.tensor.matmul(
                    st[:, 512:512 + w1], kvt[64:128, g1 * 128:(g1 + 1) * 128],
                    qt[64:128, c * 512 + off1: c * 512 + off1 + w1],
                    start=True, stop=True,
                )
                pt = ptp.tile([128, 1024], BF16, name="pt")
                if p == 0:
                    # dead-slot kill: bias -30000 on j=0 cores, 0 on j=1
                    nc.scalar.activation(pt[:, 0:512], st[:, 0:512],
                                         EXPF, bias=bqf[:, 1:2], scale=SCALE)
                    nc.scalar.activation(pt[:, 512:512 + w1], st[:, 512:512 + w1],
                                         EXPF, bias=0.0, scale=SCALE)
                elif w0 == 512:
                    nc.scalar.activation(pt[:, 0:512 + w1], st[:, 0:512 + w1],
                                         EXPF, bias=0.0, scale=SCALE)
                else:
                    nc.scalar.activation(pt[:, 0:w0], st[:, 0:w0],
                                         EXPF, bias=0.0, scale=SCALE)
                    nc.scalar.activation(pt[:, 512:512 + w1], st[:, 512:512 + w1],
                                         EXPF, bias=0.0, scale=SCALE)
                if p >= npairs - 4:
                    # odd member of the last four pairs is causal-diagonal
                    nc.vector.tensor_mul(
                        pt[:, 512:512 + w1], pt[:, 512:512 + w1], mc[:, 0:w1]
                    )
                pending.append((p, pt, (off0, w0, off1, w1)))

            def emit_av(first):
                p, pt, (off0, w0, off1, w1) = pending.pop(0)
                nc.tensor.matmul(
                    av[:, off0:off0 + w0], vs[:, 2 * p * 65:(2 * p + 1) * 65],
                    pt[:, 0:w0],
                    start=first, stop=False,
                )
                nc.tensor.matmul(
                    av[:, off1:off1 + w1],
                    vs[:, (2 * p + 1) * 65:(2 * p + 2) * 65],
                    pt[:, 512:512 + w1],
                    start=False, stop=(p == npairs - 1),
                )

            budget = 0.0
            for p in range(npairs):
                need = {("B", (2 * p + 1) // 4), ("Vt", (2 * p + 1) // 4)}
                if p == 0:
                    need |= {("A", 2 * c), ("A", 2 * c + 1)}
                ensure(need)
                emit_scores(p)
                budget += per_pair
                if late and p >= 1:
                    lsc, leng = late.pop(0)
                    trans(lsc, leng)
                while fillers and budget >= 1.0:
                    _, fn = fillers.pop(0)
                    fn()
                    budget -= 1.0
                if p >= 1:
                    emit_av(p == 1)
            emit_av(npairs == 1)
            oc = ocp.tile([65, 512], F32, name="oc")
            nc.vector.tensor_copy(oc[:], av[:])
            nc.sync.dma_start(o[c, :, :], oc[:])

        for _rep in range(repeats):
            # prologue: chunks 0,1 projected up front (B0 split for latency)
            for _, fn in (passB_units(0, split=True) + passA_units(0)
                          + vtrans_units(0) + passB_units(1)
                          + passA_units(1) + vtrans_units(1)):
                fn()

            f0 = (passB_units(2) + vtrans_units(2) + passA_units(2)
                  + passB_units(3) + vtrans_units(3) + passA_units(3))
            attn_seg(0, f0, per_pair=8.0)
            f1 = (f0 + passB_units(4) + vtrans_units(4) + passA_units(4)
                  + passB_units(5) + vtrans_units(5) + passA_units(5))
            attn_seg(1, f1, per_pair=4.0)
            f2 = (f1 + passB_units(6) + vtrans_units(6) + passA_units(6)
                  + passA_units(7))
            attn_seg(2, f2, per_pair=2.0)
            f3 = f2 + passB_units(7) + vtrans_units(7)
            attn_seg(3, f3, per_pair=1.5)
            for _, fn in f3:
                fn()

    nc.compile()
    return nc


def _get_nc():
    global _CACHED_NC
    if _CACHED_NC is None:
        _CACHED_NC = build_nc()
    return _CACHED_NC


def _host_inputs(x, wq, bq, wk, bk, wv, bv):
    bf = ml_dtypes.bfloat16
    wT = np.ascontiguousarray(
        np.concatenate([wv, wk, wq], axis=1).T
    ).astype(bf)  # [192, 768]
    xbf = np.ascontiguousarray(x).astype(bf)

    def mconst(dead):
        m = np.zeros((128, 592), np.float32)
        m[:, 0:128] = np.triu(np.ones((128, 128), np.float32))
        m[:, 128:512] = 1.0
        m[0:64, 512:576] = np.eye(64, dtype=np.float32)
        m[64:128, 576] = bq
        m[:, 577] = dead
        return np.ascontiguousarray(m.T).astype(bf)  # [592, 128]

    mT0, mT1 = mconst(-30000.0), mconst(0.0)
    in_maps = []
    for core in range(8):
        b, j = core // 2, core % 2
        if j == 0:
            xdev = np.concatenate(
                [np.zeros((128, DIN), bf), xbf[b][: SEQ - 128]], axis=0
            )
        else:
            xdev = xbf[b]
        in_maps.append({
            "x": np.ascontiguousarray(xdev),
            "wT": wT, "mT": mT0 if j == 0 else mT1,
        })
    return in_maps


def _assemble(results, bv):
    out = np.empty((4, SEQ, DOUT), np.float32)
    for core in range(8):
        b, j = core // 2, core % 2
        od = results[core]["o"]  # [NQC, 65, 512]
        for c in range(NQC):
            num = od[c, 0:64, :].astype(np.float64)
            den = od[c, 64, :].astype(np.float64)
            oc = (num / den).T.astype(np.float32) + bv  # [512, 64]
            for t in range(4):
                r0 = (8 * c + 2 * t + j) * 128
                out[b, r0:r0 + 128] = oc[t * 128:(t + 1) * 128]
    return out


def kernel(x, wq, bq, wk, bk, wv, bv):
    x = np.asarray(x, dtype=np.float32)
    args = [np.asarray(a, dtype=np.float32) for a in (wq, bq, wk, bk, wv, bv)]
    nc = _get_nc()
    in_maps = _host_inputs(x, *args)
    br = run_bass_kernel_spmd(nc, in_maps, core_ids=list(range(8)))
    return _assemble(br.results, args[5])


# revision 15
# speedup vs baseline: 1.3761x; 1.0341x over previous
"""Trainium2 Bass kernel: causal attention (QKV projection + causal softmax + AV).

Problem: x[4, 4096, 768] fp32, per-head projections to d=64, full causal
attention per batch, output [4, 4096, 64] fp32.

Sharding: 8 cores = 4 batches x 2 parity groups. Core (b, j) computes the
output rows of batch b whose 128-row block index i satisfies i % 2 == j.
One uniform SPMD program: for j=0 cores the host shifts x down by one
128-row block (prepending zeros), which makes the causal structure of both
parities identical in device coordinates (device q-blocks are always the odd
blocks 1,3,...,31; k-slot g holds true block g-1 for j=0 and g for j=1; the
dead slot 0 of j=0 is killed with a per-core -30000 exp bias).

Math shortcuts: bk is dropped (adds a per-row constant to scores ->
softmax-invariant); bv is added on the host after normalization
(sum of softmax weights is 1). Only bq is applied on device.

Device pipeline per core (all matmuls bf16, fp32 PSUM accumulation):
  proj: DMA-transposed x^T chunks (issued across 4 DGE queues) feed
     passB ([wv|wk] stationary -> V rows 0:64 / K rows 64:128 of PSUM;
     K copied to SBUF hi-half by Pool, V PE-transposed into V' = [V | 1])
     and passA (wq stationary -> Q^T in partitions 64:128 + bq via DVE).
  attn (per 512-col q chunk): per k-slot pair, two matmuls K^T_g.T @ Q^T
     into a [128, 1024] PSUM tile; exp on ACT (scale 1/8, per-core dead-slot
     bias, bf16 out); causal-diagonal mask mul on DVE; AV accumulates
     V'.T @ P^T into a [65, 512] PSUM tile whose row 64 is the softmax
     denominator. Scores run one pair ahead of AV and projection matmuls
     fill remaining PE slack so the PE never idles (keeps max p-state).
The unnormalized [65, 512] tiles go to DRAM; the host divides, adds bv,
and transposes.
"""

import numpy as np
import ml_dtypes
from contextlib import ExitStack

import concourse.bass as bass
import concourse.mybir as mybir
import concourse.tile as tile
from concourse import bacc
from concourse.bass_utils import run_bass_kernel_spmd

F32 = mybir.dt.float32
BF16 = mybir.dt.bfloat16

SEQ = 4096
DIN = 768
DOUT = 64
NCC = DIN // 128          # 6 contraction chunks
NSC = SEQ // 512          # 8 seq chunks (projection granularity)
NBLK = SEQ // 128         # 32 k-slots
NQC = 4                   # q chunks of 512 local columns (2048 own q rows)
SCALE = 1.0 / 8.0
EXPF = mybir.ActivationFunctionType.Exp

_CACHED_NC = None


def build_nc(repeats=1):
    nc = bacc.Bacc("TRN2", target_bir_lowering=False, debug=False)

    x = nc.dram_tensor("x", [SEQ, DIN], BF16, kind="ExternalInput")
    # weights pre-transposed on host: rows = [wv|wk|wq] out-cols (192)
    wT = nc.dram_tensor("wT", [192, DIN], BF16, kind="ExternalInput")
    # misc consts pre-transposed: [mask(512) | idn(64) | bq(1) | dead(1) | pad]
    mT = nc.dram_tensor("mT", [592, 128], BF16, kind="ExternalInput")
    o = nc.dram_tensor("o", [NQC, 65, 512], BF16, kind="ExternalOutput")

    with tile.TileContext(nc) as tc, ExitStack() as ctx:
        cpool = ctx.enter_context(tc.tile_pool(name="const", bufs=1))
        ptp = ctx.enter_context(tc.tile_pool(name="pt", bufs=3))
        ocp = ctx.enter_context(tc.tile_pool(name="oc", bufs=2))
        psproj = ctx.enter_context(tc.tile_pool(name="psproj", bufs=2, space="PSUM"))
        psst = ctx.enter_context(tc.tile_pool(name="psst", bufs=2, space="PSUM"))
        psav = ctx.enter_context(tc.tile_pool(name="psav", bufs=2, space="PSUM"))

        wsb = cpool.tile([128, NCC * 192], BF16)    # [wv|wk|wq] per cc chunk
        mc = cpool.tile([128, 592], BF16)           # mask | idn | bq | dead
        kvt = cpool.tile([128, NSC * 512], BF16)    # K^T rows 64:128, V^T rows 0:64
        xtf = cpool.tile([128, NSC * NCC * 512], BF16)  # x^T, whole sequence
        qt = cpool.tile([128, 16 * 128], BF16)      # Q^T own blocks, rows 64:128
        vs = cpool.tile([128, NBLK * 65], BF16)     # V' = [V | 1] per k-slot

        # ---- DMA issue plan: spread across 4 DGE queues so transfers
        # overlap and the first x^T chunk lands ~2us in.
        def trans(sc, eng, half=None):
            """DMA-transpose x rows of chunk sc (or half of it) into x^T."""
            if half is None:
                r0, r1, c0 = sc * 512, (sc + 1) * 512, 0
            else:
                r0 = sc * 512 + half * 256
                r1 = r0 + 256
                c0 = half * 256
            cols = r1 - r0
            eng.dma_start_transpose(
                xtf[:, sc * NCC * 512:(sc + 1) * NCC * 512]
                .rearrange("p (cc s) -> p cc s", cc=NCC)[:, :, c0:c0 + cols],
                x[r0:r1, :],
            )

        # All loads are DmaTransposeAnt on the single sync (SP) queue: the
        # tile framework chains every DMA issue to the previous DMA's
        # completion EXCEPT consecutive same-kind same-queue DMAs, which
        # pipeline back-to-back on the (serial) DMA engine device.
        wv3 = wsb[:].rearrange("p (cc m) -> p cc m", cc=NCC)
        nc.sync.dma_start_transpose(wv3[:, :, 0:128], wT[0:128, :])
        trans(0, nc.sync, half=0)
        trans(0, nc.sync, half=1)
        nc.sync.dma_start_transpose(wv3[:, :, 128:192], wT[128:192, :])
        nc.sync.dma_start_transpose(mc[:], mT[:, :])
        for _sc in range(1, NSC):
            trans(_sc, nc.sync)
        # f32 copies of bq / dead-slot bias (tensor_scalar needs f32 scalars)
        bqf = cpool.tile([128, 2], F32)
        nc.gpsimd.tensor_copy(bqf[:], mc[:, 576:578])
        # ones column of V'
        nc.vector.memset(
            vs[:].rearrange("p (g e) -> p g e", g=NBLK)[:, :, 64:65], 1.0
        )

        def xts(sc, cc):
            base = sc * NCC * 512 + cc * 512
            return xtf[:, base:base + 512]

        # ---- projection emitters; each returns a list of closures, one
        # per PE instruction (posts ride on the closure that needs them).
        proj_state = {}

        def passB_units(sc, split=False):
            """K^T rows 64:128 and V^T rows 0:64, [wv|wk] stationary."""
            units = []

            def mk(cc, c0, cols, start, stop):
                def f():
                    if cc == 0 and c0 == 0:
                        proj_state[("kp", sc)] = psproj.tile(
                            [128, 512], F32, tag="proj", name="kp")
                    kp = proj_state[("kp", sc)]
                    nc.tensor.matmul(
                        kp[:, c0:c0 + cols],
                        wsb[:, cc * 192:cc * 192 + 128],
                        xts(sc, cc)[:, c0:c0 + cols],
                        start=start, stop=stop,
                    )
                    if stop and c0 + cols == 512:
                        kp = proj_state.pop(("kp", sc))
                        # one copy: K rows 64:128 (no bias; bk is softmax-
                        # invariant) and V rows 0:64 land where each is used
                        nc.vector.tensor_copy(
                            kvt[:, sc * 512:(sc + 1) * 512], kp[:]
                        )
                return f

            if split:
                for c0 in (0, 256):
                    for cc in range(NCC):
                        units.append((("B", sc), mk(cc, c0, 256, cc == 0, cc == NCC - 1)))
            else:
                for cc in range(NCC):
                    units.append((("B", sc), mk(cc, 0, 512, cc == 0, cc == NCC - 1)))
            return units

        def vtrans_units(sc):
            """V' blocks via PE transpose of vt, then one DVE copy to vs."""
            units = []

            def mk(t):
                def f():
                    if t == 0:
                        proj_state[("vp", sc)] = psproj.tile(
                            [128, 256], BF16, tag="proj", name="vp")
                    vp = proj_state[("vp", sc)]
                    nc.tensor.transpose(
                        vp[:, t * 64:(t + 1) * 64],
                        kvt[0:64, sc * 512 + t * 128: sc * 512 + (t + 1) * 128],
                        mc[0:64, 512:576],
                    )
                    if t == 3:
                        vp = proj_state.pop(("vp", sc))
                        nc.vector.tensor_copy(
                            vs[:].rearrange("p (g e) -> p g e", g=NBLK)[
                                :, sc * 4:(sc + 1) * 4, 0:64
                            ],
                            vp[:].rearrange("p (g e) -> p g e", g=4),
                        )
                return f

            for t in range(4):
                units.append((("Vt", sc), mk(t)))
            return units

        def passA_units(sc):
            """Q^T for own (odd) q-blocks of this chunk, into rows 64:128."""
            units = []

            def mk(cc):
                def f():
                    if cc == 0:
                        proj_state[("qp", sc)] = psproj.tile(
                            [128, 256], F32, tag="proj", name="qp")
                    qp = proj_state[("qp", sc)]
                    rhs = (
                        xts(sc, cc)
                        .rearrange("p (a b s) -> p a b s", a=2, b=2)[:, :, 1, :]
                    )
                    nc.tensor.matmul(
                        qp[64:128, :], wsb[:, cc * 192 + 128:cc * 192 + 192], rhs,
                        start=(cc == 0), stop=(cc == NCC - 1),
                    )
                    if cc == NCC - 1:
                        qp = proj_state.pop(("qp", sc))
                        nc.vector.tensor_scalar_add(
                            qt[64:128, sc * 256:(sc + 1) * 256],
                            qp[64:128, :], bqf[64:128, 0:1],
                        )
                return f

            for cc in range(NCC):
                units.append((("A", sc), mk(cc)))
            return units

        # ---- attention ----
        def slot_geom(c, g):
            s = g - (8 * c + 1)
            if s < 1:
                return 0, 512
            off = 128 * ((s + 1) // 2)
            return off, 512 - off

        def attn_seg(c, fillers, per_pair, late=None):
            """Seg c: pairs of k-slots, scores one pair ahead of AV.

            fillers: ordered [(key, fn)] of projection units interleaved for
            PE occupancy; units a pair depends on are force-drained first.
            """
            npairs = 4 * c + 4
            av = psav.tile([65, 512], F32, tag="av", name="av")
            pending = []  # [(p, pt, geom)] awaiting AV

            def ensure(keys):
                while any(k in keys for k, _ in fillers):
                    _, fn = fillers.pop(0)
                    fn()

            def emit_scores(p):
                g0, g1 = 2 * p, 2 * p + 1
                off0, w0 = slot_geom(c, g0)
                off1, w1 = slot_geom(c, g1)
                st = psst.tile([128, 1024], F32, tag="st", name="st")
                nc.tensor.matmul(
                    st[:, 0:w0], kvt[64:128, g0 * 128:(g0 + 1) * 128],
                    qt[64:128, c * 512 + off0: c * 512 + off0 + w0],
                    start=True, stop=True,
                )
                nc.tensor.matmul(
                    st[:, 512:512 + w1], kvt[64:128, g1 * 128:(g1 + 1) * 128],
                    qt[64:128, c * 512 + off1: c * 512 + off1 + w1],
                    start=True, stop=True,
                )
                pt = ptp.tile([128, 1024], BF16, name="pt")
                if p == 0:
                    # dead-slot kill: bias -30000 on j=0 cores, 0 on j=1
                    nc.scalar.activation(pt[:, 0:512], st[:, 0:512],
                                         EXPF, bias=bqf[:, 1:2], scale=SCALE)
                    nc.scalar.activation(pt[:, 512:512 + w1], st[:, 512:512 + w1],
                                         EXPF, bias=0.0, scale=SCALE)
                elif w0 == 512:
                    nc.scalar.activation(pt[:, 0:512 + w1], st[:, 0:512 + w1],
                                         EXPF, bias=0.0, scale=SCALE)
                else:
                    # diagonal pairs have w0 == w1: one strided-AP exp covers
                    # both live regions [0:w0] and [512:512+w1]
                    sv = st[:].rearrange("p (j w) -> p j w", j=2)[:, :, 0:w0]
                    pv = pt[:].rearrange("p (j w) -> p j w", j=2)[:, :, 0:w0]
                    nc.scalar.activation(pv, sv, EXPF, bias=0.0, scale=SCALE)
                if p >= npairs - 4:
                    # odd member of the last four pairs is causal-diagonal
                    nc.vector.tensor_mul(
                        pt[:, 512:512 + w1], pt[:, 512:512 + w1], mc[:, 0:w1]
                    )
                pending.append((p, pt, (off0, w0, off1, w1)))

            def emit_av(first):
                p, pt, (off0, w0, off1, w1) = pending.pop(0)
                nc.tensor.matmul(
                    av[:, off0:off0 + w0], vs[:, 2 * p * 65:(2 * p + 1) * 65],
                    pt[:, 0:w0],
                    start=first, stop=False,
                )
                nc.tensor.matmul(
                    av[:, off1:off1 + w1],
                    vs[:, (2 * p + 1) * 65:(2 * p + 2) * 65],
                    pt[:, 512:512 + w1],
                    start=False, stop=(p == npairs - 1),
                )

            budget = 0.0
            for p in range(npairs):
                need = {("B", (2 * p + 1) // 4), ("Vt", (2 * p + 1) // 4)}
                if p == 0:
                    need |= {("A", 2 * c), ("A", 2 * c + 1)}
                ensure(need)
                emit_scores(p)
                budget += per_pair
                if late and p >= 1:
                    lsc, leng = late.pop(0)
                    trans(lsc, leng)
                while fillers and budget >= 1.0:
                    _, fn = fillers.pop(0)
                    fn()
                    budget -= 1.0
                if p >= 1:
                    emit_av(p == 1)
            emit_av(npairs == 1)
            oc = ocp.tile([65, 512], BF16, name="oc")
            nc.vector.tensor_copy(oc[:], av[:])
            nc.sync.dma_start(o[c, :, :], oc[:])

        for _rep in range(repeats):
            # prologue: chunks 0,1 projected up front (B0 split for latency)
            for _, fn in (passB_units(0, split=True) + passA_units(0)
                          + vtrans_units(0) + passB_units(1)
                          + passA_units(1) + vtrans_units(1)):
                fn()

            f0 = (passB_units(2) + vtrans_units(2) + passA_units(2)
                  + passB_units(3) + vtrans_units(3) + passA_units(3))
            attn_seg(0, f0, per_pair=8.0)
            f1 = (f0 + passB_units(4) + vtrans_units(4) + passA_units(4)
                  + passB_units(5) + vtrans_units(5) + passA_units(5))
            attn_seg(1, f1, per_pair=4.0)
            f2 = (f1 + passB_units(6) + vtrans_units(6) + passA_units(6)
                  + passA_units(7))
            attn_seg(2, f2, per_pair=2.0)
            f3 = f2 + passB_units(7) + vtrans_units(7)
            attn_seg(3, f3, per_pair=1.5)
            for _, fn in f3:
                fn()

    nc.compile()
    return nc


def _get_nc():
    global _CACHED_NC
    if _CACHED_NC is None:
        _CACHED_NC = build_nc()
    return _CACHED_NC


def _host_inputs(x, wq, bq, wk, bk, wv, bv):
    bf = ml_dtypes.bfloat16
    wT = np.ascontiguousarray(
        np.concatenate([wv, wk, wq], axis=1).T
    ).astype(bf)  # [192, 768]
    xbf = np.ascontiguousarray(x).astype(bf)

    def mconst(dead):
        m = np.zeros((128, 592), np.float32)
        m[:, 0:128] = np.triu(np.ones((128, 128), np.float32))
        m[:, 128:512] = 1.0
        m[0:64, 512:576] = np.eye(64, dtype=np.float32)
        m[64:128, 576] = bq
        m[:, 577] = dead
        return np.ascontiguousarray(m.T).astype(bf)  # [592, 128]

    mT0, mT1 = mconst(-30000.0), mconst(0.0)
    in_maps = []
    for core in range(8):
        b, j = core // 2, core % 2
        if j == 0:
            xdev = np.concatenate(
                [np.zeros((128, DIN), bf), xbf[b][: SEQ - 128]], axis=0
            )
        else:
            xdev = xbf[b]
        in_maps.append({
            "x": np.ascontiguousarray(xdev),
            "wT": wT, "mT": mT0 if j == 0 else mT1,
        })
    return in_maps


def _assemble(results, bv):
    out = np.empty((4, SEQ, DOUT), np.float32)
    for core in range(8):
        b, j = core // 2, core % 2
        od = results[core]["o"]  # [NQC, 65, 512]
        for c in range(NQC):
            num = od[c, 0:64, :].astype(np.float64)
            den = od[c, 64, :].astype(np.float64)
            oc = (num / den).T.astype(np.float32) + bv  # [512, 64]
            for t in range(4):
                r0 = (8 * c + 2 * t + j) * 128
                out[b, r0:r0 + 128] = oc[t * 128:(t + 1) * 128]
    return out


def kernel(x, wq, bq, wk, bk, wv, bv):
    x = np.asarray(x, dtype=np.float32)
    args = [np.asarray(a, dtype=np.float32) for a in (wq, bq, wk, bk, wv, bv)]
    nc = _get_nc()
    in_maps = _host_inputs(x, *args)
    br = run_bass_kernel_spmd(nc, in_maps, core_ids=list(range(8)))
    return _assemble(br.results, args[5])


# revision 17
# speedup vs baseline: 1.3914x; 1.0111x over previous
"""Trainium2 Bass kernel: causal attention (QKV projection + causal softmax + AV).

Problem: x[4, 4096, 768] fp32, per-head projections to d=64, full causal
attention per batch, output [4, 4096, 64] fp32.

Sharding: 8 cores = 4 batches x 2 parity groups. Core (b, j) computes the
output rows of batch b whose 128-row block index i satisfies i % 2 == j.
One uniform SPMD program: for j=0 cores the host shifts x down by one
128-row block (prepending zeros), which makes the causal structure of both
parities identical in device coordinates (device q-blocks are always the odd
blocks 1,3,...,31; k-slot g holds true block g-1 for j=0 and g for j=1; the
dead slot 0 of j=0 is killed with a per-core -30000 exp bias).

Math shortcuts: bk is dropped (adds a per-row constant to scores ->
softmax-invariant); bv is added on the host after normalization
(sum of softmax weights is 1). Only bq is applied on device.

Device pipeline per core (all matmuls bf16, fp32 PSUM accumulation):
  proj: DMA-transposed x^T chunks (issued across 4 DGE queues) feed
     passB ([wv|wk] stationary -> V rows 0:64 / K rows 64:128 of PSUM;
     K copied to SBUF hi-half by Pool, V PE-transposed into V' = [V | 1])
     and passA (wq stationary -> Q^T in partitions 64:128 + bq via DVE).
  attn (per 512-col q chunk): per k-slot pair, two matmuls K^T_g.T @ Q^T
     into a [128, 1024] PSUM tile; exp on ACT (scale 1/8, per-core dead-slot
     bias, bf16 out); causal-diagonal mask mul on DVE; AV accumulates
     V'.T @ P^T into a [65, 512] PSUM tile whose row 64 is the softmax
     denominator. Scores run one pair ahead of AV and projection matmuls
     fill remaining PE slack so the PE never idles (keeps max p-state).
The unnormalized [65, 512] tiles go to DRAM; the host divides, adds bv,
and transposes.
"""

import numpy as np
import ml_dtypes
from contextlib import ExitStack

import concourse.bass as bass
import concourse.mybir as mybir
import concourse.tile as tile
from concourse import bacc
from concourse.bass_utils import run_bass_kernel_spmd

F32 = mybir.dt.float32
BF16 = mybir.dt.bfloat16

SEQ = 4096
DIN = 768
DOUT = 64
NCC = DIN // 128          # 6 contraction chunks
NSC = SEQ // 512          # 8 seq chunks (projection granularity)
NBLK = SEQ // 128         # 32 k-slots
NQC = 4                   # q chunks of 512 local columns (2048 own q rows)
SCALE = 1.0 / 8.0
EXPF = mybir.ActivationFunctionType.Exp

_CACHED_NC = None


def build_nc(repeats=1):
    nc = bacc.Bacc("TRN2", target_bir_lowering=False, debug=False)

    x = nc.dram_tensor("x", [SEQ, DIN], BF16, kind="ExternalInput")
    # weights pre-transposed on host: rows = [wv|wk|wq] out-cols (192)
    wT = nc.dram_tensor("wT", [192, DIN], BF16, kind="ExternalInput")
    # misc consts pre-transposed: [mask(512) | idn(64) | bq(1) | dead(1) | pad]
    mT = nc.dram_tensor("mT", [592, 128], BF16, kind="ExternalInput")
    o = nc.dram_tensor("o", [NQC, 65, 512], BF16, kind="ExternalOutput")

    with tile.TileContext(nc) as tc, ExitStack() as ctx:
        cpool = ctx.enter_context(tc.tile_pool(name="const", bufs=1))
        ptp = ctx.enter_context(tc.tile_pool(name="pt", bufs=3))
        ocp = ctx.enter_context(tc.tile_pool(name="oc", bufs=2))
        psproj = ctx.enter_context(tc.tile_pool(name="psproj", bufs=2, space="PSUM"))
        psst = ctx.enter_context(tc.tile_pool(name="psst", bufs=2, space="PSUM"))
        psav = ctx.enter_context(tc.tile_pool(name="psav", bufs=2, space="PSUM"))

        wsb = cpool.tile([128, NCC * 192], BF16)    # [wv|wk|wq] per cc chunk
        mc = cpool.tile([128, 592], BF16)           # mask | idn | bq | dead
        kvt = cpool.tile([128, NSC * 512], BF16)    # K^T rows 64:128, V^T rows 0:64
        xtf = cpool.tile([128, NSC * NCC * 512], BF16)  # x^T, whole sequence
        qt = cpool.tile([128, 16 * 128], BF16)      # Q^T own blocks, rows 64:128
        vs = cpool.tile([128, NBLK * 65], BF16)     # V' = [V | 1] per k-slot

        # ---- DMA issue plan: spread across 4 DGE queues so transfers
        # overlap and the first x^T chunk lands ~2us in.
        def trans(sc, eng, half=None, nsplit=2):
            """DMA-transpose x rows of chunk sc (or a 1/nsplit piece)."""
            if half is None:
                r0, r1, c0 = sc * 512, (sc + 1) * 512, 0
            else:
                w = 512 // nsplit
                r0 = sc * 512 + half * w
                r1 = r0 + w
                c0 = half * w
            cols = r1 - r0
            eng.dma_start_transpose(
                xtf[:, sc * NCC * 512:(sc + 1) * NCC * 512]
                .rearrange("p (cc s) -> p cc s", cc=NCC)[:, :, c0:c0 + cols],
                x[r0:r1, :],
            )

        # All loads are DmaTransposeAnt on the single sync (SP) queue: the
        # tile framework chains every DMA issue to the previous DMA's
        # completion EXCEPT consecutive same-kind same-queue DMAs, which
        # pipeline back-to-back on the (serial) DMA engine device.
        wv3 = wsb[:].rearrange("p (cc m) -> p cc m", cc=NCC)
        nc.sync.dma_start_transpose(wv3[:, :, 0:128], wT[0:128, :])
        for _q in range(4):
            trans(0, nc.sync, half=_q, nsplit=4)
        nc.sync.dma_start_transpose(wv3[:, :, 128:192], wT[128:192, :])
        nc.sync.dma_start_transpose(mc[:], mT[:, :])
        trans(1, nc.sync, half=0)
        trans(1, nc.sync, half=1)
        for _sc in range(2, NSC):
            trans(_sc, nc.sync)
        # f32 copy of bq (tensor_scalar needs f32 scalars)
        bqf = cpool.tile([128, 2], F32)
        nc.gpsimd.tensor_copy(bqf[:], mc[:, 576:578])
        # ones column of V'; slot 0 gets a per-core 0/1 (mc col 578) which
        # zeroes the j=0 dead slot's numerator AND denominator contribution
        nc.vector.memset(
            vs[:].rearrange("p (g e) -> p g e", g=NBLK)[:, :, 64:65], 1.0
        )
        nc.vector.tensor_copy(
            vs[:].rearrange("p (g e) -> p g e", g=NBLK)[:, 0:1, 64:65],
            mc[:, 578:579],
        )

        def xts(sc, cc):
            base = sc * NCC * 512 + cc * 512
            return xtf[:, base:base + 512]

        # ---- projection emitters; each returns a list of closures, one
        # per PE instruction (posts ride on the closure that needs them).
        proj_state = {}

        def passB_units(sc, split=False):
            """K^T rows 64:128 and V^T rows 0:64, [wv|wk] stationary."""
            units = []

            def mk(cc, c0, cols, start, stop):
                def f():
                    if cc == 0 and c0 == 0:
                        proj_state[("kp", sc)] = psproj.tile(
                            [128, 512], F32, tag="proj", name="kp")
                    kp = proj_state[("kp", sc)]
                    nc.tensor.matmul(
                        kp[:, c0:c0 + cols],
                        wsb[:, cc * 192:cc * 192 + 128],
                        xts(sc, cc)[:, c0:c0 + cols],
                        start=start, stop=stop,
                    )
                    if stop and c0 + cols == 512:
                        kp = proj_state.pop(("kp", sc))
                        # one copy: K rows 64:128 (no bias; bk is softmax-
                        # invariant) and V rows 0:64 land where each is used
                        nc.vector.tensor_copy(
                            kvt[:, sc * 512:(sc + 1) * 512], kp[:]
                        )
                return f

            if split:
                w = 512 // split
                for c0 in range(0, 512, w):
                    for cc in range(NCC):
                        units.append((("B", sc), mk(cc, c0, w, cc == 0, cc == NCC - 1)))
            else:
                for cc in range(NCC):
                    units.append((("B", sc), mk(cc, 0, 512, cc == 0, cc == NCC - 1)))
            return units

        def vtrans_units(sc):
            """V' blocks via PE transpose of vt, then one DVE copy to vs."""
            units = []

            def mk(t):
                def f():
                    if t == 0:
                        proj_state[("vp", sc)] = psproj.tile(
                            [128, 256], BF16, tag="proj", name="vp")
                    vp = proj_state[("vp", sc)]
                    nc.tensor.transpose(
                        vp[:, t * 64:(t + 1) * 64],
                        kvt[0:64, sc * 512 + t * 128: sc * 512 + (t + 1) * 128],
                        mc[0:64, 512:576],
                    )
                    if t == 3:
                        vp = proj_state.pop(("vp", sc))
                        nc.vector.tensor_copy(
                            vs[:].rearrange("p (g e) -> p g e", g=NBLK)[
                                :, sc * 4:(sc + 1) * 4, 0:64
                            ],
                            vp[:].rearrange("p (g e) -> p g e", g=4),
                        )
                return f

            for t in range(4):
                units.append((("Vt", sc), mk(t)))
            return units

        def passA_units(sc):
            """Q^T for own (odd) q-blocks of this chunk, into rows 64:128."""
            units = []

            def mk(cc):
                def f():
                    if cc == 0:
                        proj_state[("qp", sc)] = psproj.tile(
                            [128, 256], F32, tag="proj", name="qp")
                    qp = proj_state[("qp", sc)]
                    rhs = (
                        xts(sc, cc)
                        .rearrange("p (a b s) -> p a b s", a=2, b=2)[:, :, 1, :]
                    )
                    nc.tensor.matmul(
                        qp[64:128, :], wsb[:, cc * 192 + 128:cc * 192 + 192], rhs,
                        start=(cc == 0), stop=(cc == NCC - 1),
                    )
                    if cc == NCC - 1:
                        qp = proj_state.pop(("qp", sc))
                        nc.vector.tensor_scalar_add(
                            qt[64:128, sc * 256:(sc + 1) * 256],
                            qp[64:128, :], bqf[64:128, 0:1],
                        )
                return f

            for cc in range(NCC):
                units.append((("A", sc), mk(cc)))
            return units

        # ---- attention ----
        def slot_geom(c, g):
            s = g - (8 * c + 1)
            if s < 1:
                return 0, 512
            off = 128 * ((s + 1) // 2)
            return off, 512 - off

        def attn_seg(c, fillers, per_pair, late=None):
            """Seg c: pairs of k-slots, scores one pair ahead of AV.

            fillers: ordered [(key, fn)] of projection units interleaved for
            PE occupancy; units a pair depends on are force-drained first.
            """
            npairs = 4 * c + 4
            av = psav.tile([65, 512], F32, tag="av", name="av")
            pending = []  # [(p, pt, geom)] awaiting AV

            def ensure(keys):
                while any(k in keys for k, _ in fillers):
                    _, fn = fillers.pop(0)
                    fn()

            def emit_scores(p):
                g0, g1 = 2 * p, 2 * p + 1
                off0, w0 = slot_geom(c, g0)
                off1, w1 = slot_geom(c, g1)
                st = psst.tile([128, 1024], F32, tag="st", name="st")
                nc.tensor.matmul(
                    st[:, 0:w0], kvt[64:128, g0 * 128:(g0 + 1) * 128],
                    qt[64:128, c * 512 + off0: c * 512 + off0 + w0],
                    start=True, stop=True,
                )
                nc.tensor.matmul(
                    st[:, 512:512 + w1], kvt[64:128, g1 * 128:(g1 + 1) * 128],
                    qt[64:128, c * 512 + off1: c * 512 + off1 + w1],
                    start=True, stop=True,
                )
                pt = ptp.tile([128, 1024], BF16, name="pt")
                if w0 == 512:
                    nc.scalar.activation(pt[:, 0:512 + w1], st[:, 0:512 + w1],
                                         EXPF, bias=0.0, scale=SCALE)
                else:
                    # diagonal pairs have w0 == w1: one strided-AP exp covers
                    # both live regions [0:w0] and [512:512+w1]
                    sv = st[:].rearrange("p (j w) -> p j w", j=2)[:, :, 0:w0]
                    pv = pt[:].rearrange("p (j w) -> p j w", j=2)[:, :, 0:w0]
                    nc.scalar.activation(pv, sv, EXPF, bias=0.0, scale=SCALE)
                if p >= npairs - 4:
                    # odd member of the last four pairs is causal-diagonal
                    nc.vector.tensor_mul(
                        pt[:, 512:512 + w1], pt[:, 512:512 + w1], mc[:, 0:w1]
                    )
                pending.append((p, pt, (off0, w0, off1, w1)))

            def emit_av(first):
                p, pt, (off0, w0, off1, w1) = pending.pop(0)
                nc.tensor.matmul(
                    av[:, off0:off0 + w0], vs[:, 2 * p * 65:(2 * p + 1) * 65],
                    pt[:, 0:w0],
                    start=first, stop=False,
                )
                nc.tensor.matmul(
                    av[:, off1:off1 + w1],
                    vs[:, (2 * p + 1) * 65:(2 * p + 2) * 65],
                    pt[:, 512:512 + w1],
                    start=False, stop=(p == npairs - 1),
                )

            budget = 0.0
            for p in range(npairs):
                need = {("B", (2 * p + 1) // 4), ("Vt", (2 * p + 1) // 4)}
                if p == 0:
                    need |= {("A", 2 * c), ("A", 2 * c + 1)}
                ensure(need)
                emit_scores(p)
                budget += per_pair
                if late and p >= 1:
                    lsc, leng = late.pop(0)
                    trans(lsc, leng)
                while fillers and budget >= 1.0:
                    _, fn = fillers.pop(0)
                    fn()
                    budget -= 1.0
                if p >= 1:
                    emit_av(p == 1)
            emit_av(npairs == 1)
            oc = ocp.tile([65, 512], BF16, name="oc")
            nc.vector.tensor_copy(oc[:], av[:])
            nc.sync.dma_start(o[c, :, :], oc[:])

        for _rep in range(repeats):
            # prologue: chunks 0,1 projected up front (B0 split for latency)
            for _, fn in (passB_units(0, split=4) + passA_units(0)
                          + vtrans_units(0) + passB_units(1, split=2)
                          + passA_units(1) + vtrans_units(1)):
                fn()

            f0 = (passB_units(2) + vtrans_units(2) + passA_units(2)
                  + passB_units(3) + vtrans_units(3) + passA_units(3))
            attn_seg(0, f0, per_pair=8.0)
            f1 = (f0 + passB_units(4) + vtrans_units(4) + passA_units(4)
                  + passB_units(5) + vtrans_units(5) + passA_units(5))
            attn_seg(1, f1, per_pair=4.0)
            f2 = (f1 + passB_units(6) + vtrans_units(6) + passA_units(6)
                  + passA_units(7))
            attn_seg(2, f2, per_pair=2.0)
            f3 = f2 + passB_units(7) + vtrans_units(7)
            attn_seg(3, f3, per_pair=0.8)
            for _, fn in f3:
                fn()

    nc.compile()
    return nc


def _get_nc():
    global _CACHED_NC
    if _CACHED_NC is None:
        _CACHED_NC = build_nc()
    return _CACHED_NC


def _host_inputs(x, wq, bq, wk, bk, wv, bv):
    bf = ml_dtypes.bfloat16
    wT = np.ascontiguousarray(
        np.concatenate([wv, wk, wq], axis=1).T
    ).astype(bf)  # [192, 768]
    xbf = np.ascontiguousarray(x).astype(bf)

    def mconst(live0):
        m = np.zeros((128, 592), np.float32)
        m[:, 0:128] = np.triu(np.ones((128, 128), np.float32))
        m[:, 128:512] = 1.0
        m[0:64, 512:576] = np.eye(64, dtype=np.float32)
        m[64:128, 576] = bq
        m[:, 578] = live0  # V' slot-0 ones column: 0 kills j=0's dead slot
        return np.ascontiguousarray(m.T).astype(bf)  # [592, 128]

    mT0, mT1 = mconst(0.0), mconst(1.0)
    in_maps = []
    for core in range(8):
        b, j = core // 2, core % 2
        if j == 0:
            xdev = np.concatenate(
                [np.zeros((128, DIN), bf), xbf[b][: SEQ - 128]], axis=0
            )
        else:
            xdev = xbf[b]
        in_maps.append({
            "x": np.ascontiguousarray(xdev),
            "wT": wT, "mT": mT0 if j == 0 else mT1,
        })
    return in_maps


def _assemble(results, bv):
    out = np.empty((4, SEQ, DOUT), np.float32)
    for core in range(8):
        b, j = core // 2, core % 2
        od = results[core]["o"]  # [NQC, 65, 512]
        for c in range(NQC):
            num = od[c, 0:64, :].astype(np.float64)
            den = od[c, 64, :].astype(np.float64)
            oc = (num / den).T.astype(np.float32) + bv  # [512, 64]
            for t in range(4):
                r0 = (8 * c + 2 * t + j) * 128
                out[b, r0:r0 + 128] = oc[t * 128:(t + 1) * 128]
    return out


def kernel(x, wq, bq, wk, bk, wv, bv):
    x = np.asarray(x, dtype=np.float32)
    args = [np.asarray(a, dtype=np.float32) for a in (wq, bq, wk, bk, wv, bv)]
    nc = _get_nc()
    in_maps = _host_inputs(x, *args)
    br = run_bass_kernel_spmd(nc, in_maps, core_ids=list(range(8)))
    return _assemble(br.results, args[5])


# revision 28
# speedup vs baseline: 1.3968x; 1.0038x over previous
"""Trainium2 Bass kernel: causal attention (QKV projection + causal softmax + AV).

Problem: x[4, 4096, 768] fp32, per-head projections to d=64, full causal
attention per batch, output [4, 4096, 64] fp32.

Sharding: 8 cores = 4 batches x 2 parity groups. Core (b, j) computes the
output rows of batch b whose 128-row block index i satisfies i % 2 == j.
One uniform SPMD program: for j=0 cores the host shifts x down by one
128-row block (prepending zeros), which makes the causal structure of both
parities identical in device coordinates (device q-blocks are always the odd
blocks 1,3,...,31; k-slot g holds true block g-1 for j=0 and g for j=1; the
dead slot 0 of j=0 is killed with a per-core -30000 exp bias).

Math shortcuts: bk is dropped (adds a per-row constant to scores ->
softmax-invariant); bv is added on the host after normalization
(sum of softmax weights is 1). Only bq is applied on device.

Device pipeline per core (all matmuls bf16, fp32 PSUM accumulation):
  proj: DMA-transposed x^T chunks (issued across 4 DGE queues) feed
     passB ([wv|wk] stationary -> V rows 0:64 / K rows 64:128 of PSUM;
     K copied to SBUF hi-half by Pool, V PE-transposed into V' = [V | 1])
     and passA (wq stationary -> Q^T in partitions 64:128 + bq via DVE).
  attn (per 512-col q chunk): per k-slot pair, two matmuls K^T_g.T @ Q^T
     into a [128, 1024] PSUM tile; exp on ACT (scale 1/8, per-core dead-slot
     bias, bf16 out); causal-diagonal mask mul on DVE; AV accumulates
     V'.T @ P^T into a [65, 512] PSUM tile whose row 64 is the softmax
     denominator. Scores run one pair ahead of AV and projection matmuls
     fill remaining PE slack so the PE never idles (keeps max p-state).
The unnormalized [65, 512] tiles go to DRAM; the host divides, adds bv,
and transposes.
"""

import numpy as np
import ml_dtypes
from contextlib import ExitStack

import concourse.bass as bass
import concourse.mybir as mybir
import concourse.tile as tile
from concourse import bacc
from concourse.bass_utils import run_bass_kernel_spmd

F32 = mybir.dt.float32
BF16 = mybir.dt.bfloat16

SEQ = 4096
DIN = 768
DOUT = 64
NCC = DIN // 128          # 6 contraction chunks
NSC = SEQ // 512          # 8 seq chunks (projection granularity)
NBLK = SEQ // 128         # 32 k-slots
NQC = 4                   # q chunks of 512 local columns (2048 own q rows)
SCALE = 1.0 / 8.0
EXPF = mybir.ActivationFunctionType.Exp

_CACHED_NC = None


def build_nc(repeats=1):
    nc = bacc.Bacc("TRN2", target_bir_lowering=False, debug=False)

    x = nc.dram_tensor("x", [SEQ, DIN], BF16, kind="ExternalInput")
    # weights pre-transposed on host: rows = [wv|wk|wq] out-cols (192)
    wT = nc.dram_tensor("wT", [192, DIN], BF16, kind="ExternalInput")
    # misc consts pre-transposed: [mask(512) | idn(64) | bq(1) | dead(1) | pad]
    mT = nc.dram_tensor("mT", [592, 128], BF16, kind="ExternalInput")
    o = nc.dram_tensor("o", [NQC, 65, 512], BF16, kind="ExternalOutput")

    with tile.TileContext(nc) as tc, ExitStack() as ctx:
        cpool = ctx.enter_context(tc.tile_pool(name="const", bufs=1))
        ptp = ctx.enter_context(tc.tile_pool(name="pt", bufs=3))
        ocp = ctx.enter_context(tc.tile_pool(name="oc", bufs=2))
        psproj = ctx.enter_context(tc.tile_pool(name="psproj", bufs=2, space="PSUM"))
        psst = ctx.enter_context(tc.tile_pool(name="psst", bufs=2, space="PSUM"))
        psav = ctx.enter_context(tc.tile_pool(name="psav", bufs=2, space="PSUM"))

        wsb = cpool.tile([128, NCC * 192], BF16)    # [wv|wk|wq] per cc chunk
        mc = cpool.tile([128, 592], BF16)           # mask | idn | bq | dead
        kvt = cpool.tile([128, NSC * 512], BF16)    # K^T rows 64:128, V^T rows 0:64
        xtf = cpool.tile([128, NSC * NCC * 512], BF16)  # x^T, whole sequence
        qt = cpool.tile([128, 16 * 128], BF16)      # Q^T own blocks, rows 64:128
        vs = cpool.tile([128, NBLK * 65], BF16)     # V' = [V | 1] per k-slot

        # ---- DMA issue plan: spread across 4 DGE queues so transfers
        # overlap and the first x^T chunk lands ~2us in.
        def trans(sc, eng, half=None, nsplit=2):
            """DMA-transpose x rows of chunk sc (or a 1/nsplit piece)."""
            if half is None:
                r0, r1, c0 = sc * 512, (sc + 1) * 512, 0
            else:
                w = 512 // nsplit
                r0 = sc * 512 + half * w
                r1 = r0 + w
                c0 = half * w
            cols = r1 - r0
            eng.dma_start_transpose(
                xtf[:, sc * NCC * 512:(sc + 1) * NCC * 512]
                .rearrange("p (cc s) -> p cc s", cc=NCC)[:, :, c0:c0 + cols],
                x[r0:r1, :],
            )

        # All loads are DmaTransposeAnt on the single sync (SP) queue: the
        # tile framework chains every DMA issue to the previous DMA's
        # completion EXCEPT consecutive same-kind same-queue DMAs, which
        # pipeline back-to-back on the (serial) DMA engine device.
        wv3 = wsb[:].rearrange("p (cc m) -> p cc m", cc=NCC)
        nc.sync.dma_start_transpose(wv3[:, 0:1, 0:128],
                                    wT[0:128, 0:128])
        trans(0, nc.sync, half=0, nsplit=4)
        nc.sync.dma_start_transpose(wv3[:, 1:NCC, 0:128],
                                    wT[0:128, 128:DIN])
        for _q in range(1, 4):
            trans(0, nc.sync, half=_q, nsplit=4)
        nc.sync.dma_start_transpose(wv3[:, :, 128:192], wT[128:192, :])
        nc.sync.dma_start_transpose(mc[:], mT[:, :])
        trans(1, nc.sync, half=0)
        trans(1, nc.sync, half=1)
        for _sc in range(2, NSC):
            trans(_sc, nc.sync)
        # f32 copy of bq (tensor_scalar needs f32 scalars)
        bqf = cpool.tile([128, 2], F32)
        nc.gpsimd.tensor_copy(bqf[:], mc[:, 576:578])
        # ones column of V'; slot 0 gets a per-core 0/1 (mc col 578) which
        # zeroes the j=0 dead slot's numerator AND denominator contribution
        nc.vector.memset(
            vs[:].rearrange("p (g e) -> p g e", g=NBLK)[:, :, 64:65], 1.0
        )
        nc.vector.tensor_copy(
            vs[:].rearrange("p (g e) -> p g e", g=NBLK)[:, 0:1, 64:65],
            mc[:, 578:579],
        )

        def xts(sc, cc):
            base = sc * NCC * 512 + cc * 512
            return xtf[:, base:base + 512]

        # ---- projection emitters; each returns a list of closures, one
        # per PE instruction (posts ride on the closure that needs them).
        proj_state = {}

        def passB_units(sc, split=False):
            """K^T rows 64:128 and V^T rows 0:64, [wv|wk] stationary."""
            units = []

            def mk(cc, c0, cols, start, stop):
                def f():
                    if cc == 0 and c0 == 0:
                        proj_state[("kp", sc)] = psproj.tile(
                            [128, 512], F32, tag="proj", name="kp")
                    kp = proj_state[("kp", sc)]
                    nc.tensor.matmul(
                        kp[:, c0:c0 + cols],
                        wsb[:, cc * 192:cc * 192 + 128],
                        xts(sc, cc)[:, c0:c0 + cols],
                        start=start, stop=stop,
                    )
                    if stop and c0 + cols == 512:
                        kp = proj_state.pop(("kp", sc))
                        # one copy: K rows 64:128 (no bias; bk is softmax-
                        # invariant) and V rows 0:64 land where each is used
                        nc.vector.tensor_copy(
                            kvt[:, sc * 512:(sc + 1) * 512], kp[:]
                        )
                return f

            if split:
                w = 512 // split
                for c0 in range(0, 512, w):
                    for cc in range(NCC):
                        units.append((("B", sc), mk(cc, c0, w, cc == 0, cc == NCC - 1)))
            else:
                for cc in range(NCC):
                    units.append((("B", sc), mk(cc, 0, 512, cc == 0, cc == NCC - 1)))
            return units

        def vtrans_units(sc):
            """V' blocks via PE transpose of vt, then one DVE copy to vs."""
            units = []

            def mk(t):
                def f():
                    if t == 0:
                        proj_state[("vp", sc)] = psproj.tile(
                            [128, 256], BF16, tag="proj", name="vp")
                    vp = proj_state[("vp", sc)]
                    nc.tensor.transpose(
                        vp[:, t * 64:(t + 1) * 64],
                        kvt[0:64, sc * 512 + t * 128: sc * 512 + (t + 1) * 128],
                        mc[0:64, 512:576],
                    )
                    if t == 3:
                        vp = proj_state.pop(("vp", sc))
                        nc.vector.tensor_copy(
                            vs[:].rearrange("p (g e) -> p g e", g=NBLK)[
                                :, sc * 4:(sc + 1) * 4, 0:64
                            ],
                            vp[:].rearrange("p (g e) -> p g e", g=4),
                        )
                return f

            for t in range(4):
                units.append((("Vt", sc), mk(t)))
            return units

        def passA_units(sc):
            """Q^T for own (odd) q-blocks of this chunk, into rows 64:128."""
            units = []

            def mk(cc):
                def f():
                    if cc == 0:
                        proj_state[("qp", sc)] = psproj.tile(
                            [128, 256], F32, tag="proj", name="qp")
                    qp = proj_state[("qp", sc)]
                    rhs = (
                        xts(sc, cc)
                        .rearrange("p (a b s) -> p a b s", a=2, b=2)[:, :, 1, :]
                    )
                    nc.tensor.matmul(
                        qp[64:128, :], wsb[:, cc * 192 + 128:cc * 192 + 192], rhs,
                        start=(cc == 0), stop=(cc == NCC - 1),
                    )
                    if cc == NCC - 1:
                        qp = proj_state.pop(("qp", sc))
                        nc.vector.tensor_scalar_add(
                            qt[64:128, sc * 256:(sc + 1) * 256],
                            qp[64:128, :], bqf[64:128, 0:1],
                        )
                return f

            for cc in range(NCC):
                units.append((("A", sc), mk(cc)))
            return units

        # ---- attention ----
        def slot_geom(c, g):
            s = g - (8 * c + 1)
            if s < 1:
                return 0, 512
            off = 128 * ((s + 1) // 2)
            return off, 512 - off

        parts = {}

        def attn_seg(c, p_lo, p_hi, final, fillers, per_pair):
            """Seg c pairs [p_lo, p_hi): scores one pair ahead of AV.

            fillers: ordered [(key, fn)] of projection units interleaved for
            PE occupancy; units a pair depends on are force-drained first.
            Non-final ranges park their partial AV in SBUF (parts).
            """
            npairs = 4 * c + 4
            av = psav.tile([65, 512], F32, tag="av", name="av")
            pending = []  # [(p, pt, geom)] awaiting AV

            def ensure(keys):
                while any(k in keys for k, _ in fillers):
                    _, fn = fillers.pop(0)
                    fn()

            def emit_scores(p):
                g0, g1 = 2 * p, 2 * p + 1
                off0, w0 = slot_geom(c, g0)
                off1, w1 = slot_geom(c, g1)
                st = psst.tile([128, 1024], F32, tag="st", name="st")
                if c == 0 and p < 2:
                    # chunk-1 Q lands late: columns [0:256] (chunk-0 Q) can
                    # score (and exp) before passA(1) completes
                    pt = ptp.tile([128, 1024], BF16, name="pt")
                    for cl, ch in ((0, 256), (256, 512)):
                        for g, base in ((g0, 0), (g1, 512)):
                            nc.tensor.matmul(
                                st[:, base + cl:base + ch],
                                kvt[64:128, g * 128:(g + 1) * 128],
                                qt[64:128, cl:ch],
                                start=True, stop=True,
                            )
                        nc.scalar.activation(
                            pt[:, cl:ch], st[:, cl:ch],
                            EXPF, bias=0.0, scale=SCALE)
                        nc.scalar.activation(
                            pt[:, 512 + cl:512 + ch], st[:, 512 + cl:512 + ch],
                            EXPF, bias=0.0, scale=SCALE)
                    nc.vector.tensor_mul(
                        pt[:, 512:512 + w1], pt[:, 512:512 + w1], mc[:, 0:w1]
                    )
                    pending.append((p, pt, (off0, w0, off1, w1)))
                    return
                nc.tensor.matmul(
                    st[:, 0:w0], kvt[64:128, g0 * 128:(g0 + 1) * 128],
                    qt[64:128, c * 512 + off0: c * 512 + off0 + w0],
                    start=True, stop=True,
                )
                nc.tensor.matmul(
                    st[:, 512:512 + w1], kvt[64:128, g1 * 128:(g1 + 1) * 128],
                    qt[64:128, c * 512 + off1: c * 512 + off1 + w1],
                    start=True, stop=True,
                )
                pt = ptp.tile([128, 1024], BF16, name="pt")
                if w0 == 512:
                    nc.scalar.activation(pt[:, 0:512 + w1], st[:, 0:512 + w1],
                                         EXPF, bias=0.0, scale=SCALE)
                else:
                    # diagonal pairs have w0 == w1: one strided-AP exp covers
                    # both live regions [0:w0] and [512:512+w1]
                    sv = st[:].rearrange("p (j w) -> p j w", j=2)[:, :, 0:w0]
                    pv = pt[:].rearrange("p (j w) -> p j w", j=2)[:, :, 0:w0]
                    nc.scalar.activation(pv, sv, EXPF, bias=0.0, scale=SCALE)
                if p >= npairs - 4:
                    # odd member of the last four pairs is causal-diagonal
                    nc.vector.tensor_mul(
                        pt[:, 512:512 + w1], pt[:, 512:512 + w1], mc[:, 0:w1]
                    )
                pending.append((p, pt, (off0, w0, off1, w1)))

            splitL = final and c == 3 and p_hi == npairs
            started = set()

            def emit_av(first):
                p, pt, (off0, w0, off1, w1) = pending.pop(0)
                for g, off, base in ((2 * p, off0, 0),
                                     (2 * p + 1, off1, 512)):
                    vsl = vs[:, g * 65:(g + 1) * 65]
                    if not splitL:
                        nc.tensor.matmul(
                            av[:, off:512], vsl, pt[:, base:base + 512 - off],
                            start=first and base == 0,
                            stop=(p == p_hi - 1 and base == 512),
                        )
                        continue
                    # L=[0:256] closes at pair 13 (slot 27); R at pair 15
                    pieces = []
                    if off < 256:
                        pieces.append(("L", off, 256, g == 27))
                    pieces.append(("R", max(off, 256), 512, g == 31))
                    for grp, o0, o1, stop in pieces:
                        nc.tensor.matmul(
                            av[:, o0:o1], vsl,
                            pt[:, base + o0 - off: base + o1 - off],
                            start=grp not in started, stop=stop,
                        )
                        started.add(grp)
                        if stop and grp == "L":
                            ocl = ocp.tile([65, 256], BF16, name="ocl")
                            nc.vector.tensor_copy(ocl[:], av[:, 0:256])
                            nc.sync.dma_start(o[c, :, 0:256], ocl[:])

            budget = 0.0
            for p in range(p_lo, p_hi):
                need = {("B", (2 * p + 1) // 4), ("Vt", (2 * p + 1) // 4)}
                if p == p_lo:
                    need |= {("A", 2 * c), ("A", 2 * c + 1)}
                ensure(need)
                emit_scores(p)
                budget += per_pair
                while fillers and budget >= 1.0:
                    _, fn = fillers.pop(0)
                    fn()
                    budget -= 1.0
                if p >= p_lo + 1:
                    emit_av(p == p_lo + 1)
            emit_av(p_hi == p_lo + 1)
            if splitL:
                oc = ocp.tile([65, 256], BF16, name="oc")
                nc.vector.tensor_copy(oc[:], av[:, 256:512])
                nc.sync.dma_start(o[c, :, 256:512], oc[:])
            elif final:
                oc = ocp.tile([65, 512], BF16, name="oc")
                if c in parts:
                    nc.vector.tensor_add(oc[:], av[:], parts.pop(c)[:])
                else:
                    nc.vector.tensor_copy(oc[:], av[:])
                nc.sync.dma_start(o[c, :, :], oc[:])
            else:
                part = ocp.tile([65, 512], F32, tag="part", name="part")
                nc.vector.tensor_copy(part[:], av[:])
                parts[c] = part

        for _rep in range(repeats):
            # prologue: chunks 0,1 projected up front (B0 split for latency)
            for _, fn in (passB_units(0, split=4) + passA_units(0)
                          + vtrans_units(0) + passB_units(1, split=2)
                          + passA_units(1) + vtrans_units(1)):
                fn()

            f0 = (passB_units(2) + vtrans_units(2) + passA_units(2)
                  + passB_units(3) + vtrans_units(3) + passA_units(3))
            attn_seg(0, 0, 4, True, f0, per_pair=8.0)
            f1 = (f0 + passB_units(4) + vtrans_units(4) + passA_units(4)
                  + passB_units(5) + vtrans_units(5) + passA_units(5))
            attn_seg(1, 0, 8, True, f1, per_pair=4.0)
            attn_seg(2, 0, 12, True, f1, per_pair=2.0)
            f3 = (f1 + passA_units(6) + passB_units(6) + vtrans_units(6)
                  + passA_units(7) + passB_units(7) + vtrans_units(7))
            attn_seg(3, 0, 16, True, f3, per_pair=0.8)
            for _, fn in f3:
                fn()

    nc.compile()
    return nc


def _get_nc():
    global _CACHED_NC
    if _CACHED_NC is None:
        _CACHED_NC = build_nc()
    return _CACHED_NC


def _host_inputs(x, wq, bq, wk, bk, wv, bv):
    bf = ml_dtypes.bfloat16
    wT = np.ascontiguousarray(
        np.concatenate([wv, wk, wq], axis=1).T
    ).astype(bf)  # [192, 768]
    xbf = np.ascontiguousarray(x).astype(bf)

    def mconst(live0):
        m = np.zeros((128, 592), np.float32)
        m[:, 0:128] = np.triu(np.ones((128, 128), np.float32))
        m[:, 128:512] = 1.0
        m[0:64, 512:576] = np.eye(64, dtype=np.float32)
        m[64:128, 576] = bq
        m[:, 578] = live0  # V' slot-0 ones column: 0 kills j=0's dead slot
        return np.ascontiguousarray(m.T).astype(bf)  # [592, 128]

    mT0, mT1 = mconst(0.0), mconst(1.0)
    in_maps = []
    for core in range(8):
        b, j = core // 2, core % 2
        if j == 0:
            xdev = np.concatenate(
                [np.zeros((128, DIN), bf), xbf[b][: SEQ - 128]], axis=0
            )
        else:
            xdev = xbf[b]
        in_maps.append({
            "x": np.ascontiguousarray(xdev),
            "wT": wT, "mT": mT0 if j == 0 else mT1,
        })
    return in_maps


def _assemble(results, bv):
    out = np.empty((4, SEQ, DOUT), np.float32)
    for core in range(8):
        b, j = core // 2, core % 2
        od = results[core]["o"]  # [NQC, 65, 512]
        for c in range(NQC):
            num = od[c, 0:64, :].astype(np.float64)
            den = od[c, 64, :].astype(np.float64)
            oc = (num / den).T.astype(np.float32) + bv  # [512, 64]
            for t in range(4):
                r0 = (8 * c + 2 * t + j) * 128
                out[b, r0:r0 + 128] = oc[t * 128:(t + 1) * 128]
    return out


def kernel(x, wq, bq, wk, bk, wv, bv):
    x = np.asarray(x, dtype=np.float32)
    args = [np.asarray(a, dtype=np.float32) for a in (wq, bq, wk, bk, wv, bv)]
    nc = _get_nc()
    in_maps = _host_inputs(x, *args)
    br = run_bass_kernel_spmd(nc, in_maps, core_ids=list(range(8)))
    return _assemble(br.results, args[5])


# revision 35
# speedup vs baseline: 1.4055x; 1.0063x over previous
"""Trainium2 Bass kernel: causal attention (QKV projection + causal softmax + AV).

Problem: x[4, 4096, 768] fp32, per-head projections to d=64, full causal
attention per batch, output [4, 4096, 64] fp32.

Sharding: 8 cores = 4 batches x 2 parity groups. Core (b, j) computes the
output rows of batch b whose 128-row block index i satisfies i % 2 == j.
One uniform SPMD program: for j=0 cores the host shifts x down by one
128-row block (prepending zeros), which makes the causal structure of both
parities identical in device coordinates (device q-blocks are always the odd
blocks 1,3,...,31; k-slot g holds true block g-1 for j=0 and g for j=1; the
dead slot 0 of j=0 is killed with a per-core -30000 exp bias).

Math shortcuts: bk is dropped (adds a per-row constant to scores ->
softmax-invariant); bv is added on the host after normalization
(sum of softmax weights is 1). Only bq is applied on device.

Device pipeline per core (all matmuls bf16, fp32 PSUM accumulation):
  proj: DMA-transposed x^T chunks (issued across 4 DGE queues) feed
     passB ([wv|wk] stationary -> V rows 0:64 / K rows 64:128 of PSUM;
     K copied to SBUF hi-half by Pool, V PE-transposed into V' = [V | 1])
     and passA (wq stationary -> Q^T in partitions 64:128 + bq via DVE).
  attn (per 512-col q chunk): per k-slot pair, two matmuls K^T_g.T @ Q^T
     into a [128, 1024] PSUM tile; exp on ACT (scale 1/8, per-core dead-slot
     bias, bf16 out); causal-diagonal mask mul on DVE; AV accumulates
     V'.T @ P^T into a [65, 512] PSUM tile whose row 64 is the softmax
     denominator. Scores run one pair ahead of AV and projection matmuls
     fill remaining PE slack so the PE never idles (keeps max p-state).
The unnormalized [65, 512] tiles go to DRAM; the host divides, adds bv,
and transposes.
"""

import numpy as np
import ml_dtypes
from contextlib import ExitStack

import concourse.bass as bass
import concourse.mybir as mybir
import concourse.tile as tile
from concourse import bacc
from concourse.bass_utils import run_bass_kernel_spmd

F32 = mybir.dt.float32
BF16 = mybir.dt.bfloat16

SEQ = 4096
DIN = 768
DOUT = 64
NCC = DIN // 128          # 6 contraction chunks
NSC = SEQ // 512          # 8 seq chunks (projection granularity)
NBLK = SEQ // 128         # 32 k-slots
NQC = 4                   # q chunks of 512 local columns (2048 own q rows)
SCALE = 1.0 / 8.0
EXPF = mybir.ActivationFunctionType.Exp

_CACHED_NC = None


def build_nc(repeats=1):
    nc = bacc.Bacc("TRN2", target_bir_lowering=False, debug=False)

    x = nc.dram_tensor("x", [SEQ, DIN], BF16, kind="ExternalInput")
    # weights pre-transposed on host: rows = [wv|wk|wq] out-cols (192)
    wT = nc.dram_tensor("wT", [192, DIN], BF16, kind="ExternalInput")
    # misc consts pre-transposed: [mask(512) | idn(64) | bq(1) | dead(1) | pad]
    mT = nc.dram_tensor("mT", [592, 128], BF16, kind="ExternalInput")
    o = nc.dram_tensor("o", [NQC, 65, 512], BF16, kind="ExternalOutput")

    with tile.TileContext(nc) as tc, ExitStack() as ctx:
        cpool = ctx.enter_context(tc.tile_pool(name="const", bufs=1))
        ptp = ctx.enter_context(tc.tile_pool(name="pt", bufs=3))
        ocp = ctx.enter_context(tc.tile_pool(name="oc", bufs=2))
        psproj = ctx.enter_context(tc.tile_pool(name="psproj", bufs=2, space="PSUM"))
        psst = ctx.enter_context(tc.tile_pool(name="psst", bufs=2, space="PSUM"))
        psav = ctx.enter_context(tc.tile_pool(name="psav", bufs=2, space="PSUM"))

        wsb = cpool.tile([128, NCC * 192], BF16)    # [wv|wk|wq] per cc chunk
        mc = cpool.tile([128, 592], BF16)           # mask | idn | bq | dead
        kvt = cpool.tile([128, NSC * 512], BF16)    # K^T rows 64:128, V^T rows 0:64
        xtf = cpool.tile([128, NSC * NCC * 512], BF16)  # x^T, whole sequence
        qt = cpool.tile([128, 16 * 128], BF16)      # Q^T own blocks, rows 64:128
        vs = cpool.tile([128, NBLK * 65], BF16)     # V' = [V | 1] per k-slot

        # ---- DMA issue plan: spread across 4 DGE queues so transfers
        # overlap and the first x^T chunk lands ~2us in.
        def trans(sc, eng, half=None, nsplit=2):
            """DMA-transpose x rows of chunk sc (or a 1/nsplit piece)."""
            if half is None:
                r0, r1, c0 = sc * 512, (sc + 1) * 512, 0
            else:
                w = 512 // nsplit
                r0 = sc * 512 + half * w
                r1 = r0 + w
                c0 = half * w
            cols = r1 - r0
            eng.dma_start_transpose(
                xtf[:, sc * NCC * 512:(sc + 1) * NCC * 512]
                .rearrange("p (cc s) -> p cc s", cc=NCC)[:, :, c0:c0 + cols],
                x[r0:r1, :],
            )

        # All loads are DmaTransposeAnt on the single sync (SP) queue: the
        # tile framework chains every DMA issue to the previous DMA's
        # completion EXCEPT consecutive same-kind same-queue DMAs, which
        # pipeline back-to-back on the (serial) DMA engine device.
        wv3 = wsb[:].rearrange("p (cc m) -> p cc m", cc=NCC)
        nc.sync.dma_start_transpose(wv3[:, :, 0:128], wT[0:128, :])
        trans(0, nc.sync, half=0)
        trans(0, nc.sync, half=1)
        nc.sync.dma_start_transpose(wv3[:, :, 128:192], wT[128:192, :])
        nc.sync.dma_start_transpose(mc[:], mT[:, :])
        trans(1, nc.sync, half=0)
        trans(1, nc.sync, half=1)
        for _sc in range(2, NSC):
            trans(_sc, nc.sync)
        # f32 copy of bq (tensor_scalar needs f32 scalars)
        bqf = cpool.tile([128, 2], F32)
        nc.gpsimd.tensor_copy(bqf[:], mc[:, 576:578])
        # ones column of V'; slot 0 gets a per-core 0/1 (mc col 578) which
        # zeroes the j=0 dead slot's numerator AND denominator contribution
        nc.vector.memset(
            vs[:].rearrange("p (g e) -> p g e", g=NBLK)[:, :, 64:65], 1.0
        )
        nc.vector.tensor_copy(
            vs[:].rearrange("p (g e) -> p g e", g=NBLK)[:, 0:1, 64:65],
            mc[:, 578:579],
        )

        def xts(sc, cc):
            base = sc * NCC * 512 + cc * 512
            return xtf[:, base:base + 512]

        # ---- projection emitters; each returns a list of closures, one
        # per PE instruction (posts ride on the closure that needs them).
        proj_state = {}

        def passB_units(sc, split=False):
            """K^T rows 64:128 and V^T rows 0:64, [wv|wk] stationary."""
            units = []

            def mk(cc, c0, cols, start, stop):
                def f():
                    if cc == 0 and c0 == 0:
                        proj_state[("kp", sc)] = psproj.tile(
                            [128, 512], F32, tag="proj", name="kp")
                    kp = proj_state[("kp", sc)]
                    nc.tensor.matmul(
                        kp[:, c0:c0 + cols],
                        wsb[:, cc * 192:cc * 192 + 128],
                        xts(sc, cc)[:, c0:c0 + cols],
                        start=start, stop=stop,
                    )
                    if stop and c0 + cols == 512:
                        kp = proj_state.pop(("kp", sc))
                        # one copy: K rows 64:128 (no bias; bk is softmax-
                        # invariant) and V rows 0:64 land where each is used
                        nc.vector.tensor_copy(
                            kvt[:, sc * 512:(sc + 1) * 512], kp[:]
                        )
                return f

            if split:
                w = 512 // split
                for c0 in range(0, 512, w):
                    for cc in range(NCC):
                        units.append((("B", sc), mk(cc, c0, w, cc == 0, cc == NCC - 1)))
            else:
                for cc in range(NCC):
                    units.append((("B", sc), mk(cc, 0, 512, cc == 0, cc == NCC - 1)))
            return units

        def vtrans_units(sc):
            """V' blocks via PE transpose of vt, then one DVE copy to vs."""
            units = []

            def mk(t):
                def f():
                    if t == 0:
                        proj_state[("vp", sc)] = psproj.tile(
                            [128, 256], BF16, tag="proj", name="vp")
                    vp = proj_state[("vp", sc)]
                    nc.tensor.transpose(
                        vp[:, t * 64:(t + 1) * 64],
                        kvt[0:64, sc * 512 + t * 128: sc * 512 + (t + 1) * 128],
                        mc[0:64, 512:576],
                    )
                    if t == 3:
                        vp = proj_state.pop(("vp", sc))
                        nc.vector.tensor_copy(
                            vs[:].rearrange("p (g e) -> p g e", g=NBLK)[
                                :, sc * 4:(sc + 1) * 4, 0:64
                            ],
                            vp[:].rearrange("p (g e) -> p g e", g=4),
                        )
                return f

            for t in range(4):
                units.append((("Vt", sc), mk(t)))
            return units

        def passA_units(sc):
            """Q^T for own (odd) q-blocks of this chunk, into rows 64:128."""
            units = []

            def mk(cc):
                def f():
                    if cc == 0:
                        proj_state[("qp", sc)] = psproj.tile(
                            [128, 256], F32, tag="proj", name="qp")
                    qp = proj_state[("qp", sc)]
                    rhs = (
                        xts(sc, cc)
                        .rearrange("p (a b s) -> p a b s", a=2, b=2)[:, :, 1, :]
                    )
                    nc.tensor.matmul(
                        qp[64:128, :], wsb[:, cc * 192 + 128:cc * 192 + 192], rhs,
                        start=(cc == 0), stop=(cc == NCC - 1),
                    )
                    if cc == NCC - 1:
                        qp = proj_state.pop(("qp", sc))
                        nc.vector.tensor_scalar_add(
                            qt[64:128, sc * 256:(sc + 1) * 256],
                            qp[64:128, :], bqf[64:128, 0:1],
                        )
                return f

            for cc in range(NCC):
                units.append((("A", sc), mk(cc)))
            return units

        # ---- attention ----
        def slot_geom(c, g):
            s = g - (8 * c + 1)
            if s < 1:
                return 0, 512
            off = 128 * ((s + 1) // 2)
            return off, 512 - off

        parts = {}

        def attn_seg(c, p_lo, p_hi, final, fillers, per_pair):
            """Seg c pairs [p_lo, p_hi): scores one pair ahead of AV.

            fillers: ordered [(key, fn)] of projection units interleaved for
            PE occupancy; units a pair depends on are force-drained first.
            Non-final ranges park their partial AV in SBUF (parts).
            """
            npairs = 4 * c + 4
            av = psav.tile([65, 512], F32, tag="av", name="av")
            pending = []  # [(p, pt, geom)] awaiting AV

            def ensure(keys):
                while any(k in keys for k, _ in fillers):
                    _, fn = fillers.pop(0)
                    fn()

            def emit_scores(p):
                g0, g1 = 2 * p, 2 * p + 1
                off0, w0 = slot_geom(c, g0)
                off1, w1 = slot_geom(c, g1)
                st = psst.tile([128, 1024], F32, tag="st", name="st")
                if c == 0 and p < 2:
                    # the seg's second Q chunk lands late: columns [0:256]
                    # (first chunk's Q) can score and exp before the second
                    # passA completes; respects the compacted score layout
                    pt = ptp.tile([128, 1024], BF16, name="pt")
                    for cl, ch in ((0, 256), (256, 512)):
                        for g, base, off in ((g0, 0, off0), (g1, 512, off1)):
                            a = max(cl, off)
                            if a >= ch:
                                continue
                            nc.tensor.matmul(
                                st[:, base + a - off:base + ch - off],
                                kvt[64:128, g * 128:(g + 1) * 128],
                                qt[64:128, c * 512 + a:c * 512 + ch],
                                start=True, stop=True,
                            )
                            nc.scalar.activation(
                                pt[:, base + a - off:base + ch - off],
                                st[:, base + a - off:base + ch - off],
                                EXPF, bias=0.0, scale=SCALE)
                    if p >= npairs - 4:
                        nc.vector.tensor_mul(
                            pt[:, 512:512 + w1], pt[:, 512:512 + w1],
                            mc[:, 0:w1]
                        )
                    pending.append((p, pt, (off0, w0, off1, w1)))
                    return
                nc.tensor.matmul(
                    st[:, 0:w0], kvt[64:128, g0 * 128:(g0 + 1) * 128],
                    qt[64:128, c * 512 + off0: c * 512 + off0 + w0],
                    start=True, stop=True,
                )
                nc.tensor.matmul(
                    st[:, 512:512 + w1], kvt[64:128, g1 * 128:(g1 + 1) * 128],
                    qt[64:128, c * 512 + off1: c * 512 + off1 + w1],
                    start=True, stop=True,
                )
                pt = ptp.tile([128, 1024], BF16, name="pt")
                if w0 == 512:
                    nc.scalar.activation(pt[:, 0:512 + w1], st[:, 0:512 + w1],
                                         EXPF, bias=0.0, scale=SCALE)
                else:
                    # diagonal pairs have w0 == w1: one strided-AP exp covers
                    # both live regions [0:w0] and [512:512+w1]
                    sv = st[:].rearrange("p (j w) -> p j w", j=2)[:, :, 0:w0]
                    pv = pt[:].rearrange("p (j w) -> p j w", j=2)[:, :, 0:w0]
                    nc.scalar.activation(pv, sv, EXPF, bias=0.0, scale=SCALE)
                if p >= npairs - 4:
                    # odd member of the last four pairs is causal-diagonal
                    nc.vector.tensor_mul(
                        pt[:, 512:512 + w1], pt[:, 512:512 + w1], mc[:, 0:w1]
                    )
                pending.append((p, pt, (off0, w0, off1, w1)))

            splitL = final and c == 3 and p_hi == npairs
            started = set()

            def emit_av(first):
                p, pt, (off0, w0, off1, w1) = pending.pop(0)
                for g, off, base in ((2 * p, off0, 0),
                                     (2 * p + 1, off1, 512)):
                    vsl = vs[:, g * 65:(g + 1) * 65]
                    if not splitL:
                        nc.tensor.matmul(
                            av[:, off:512], vsl, pt[:, base:base + 512 - off],
                            start=first and base == 0,
                            stop=(p == p_hi - 1 and base == 512),
                        )
                        continue
                    # L=[0:256] closes at pair 13 (slot 27); R at pair 15
                    pieces = []
                    if off < 256:
                        pieces.append(("L", off, 256, g == 27))
                    pieces.append(("R", max(off, 256), 512, g == 31))
                    for grp, o0, o1, stop in pieces:
                        nc.tensor.matmul(
                            av[:, o0:o1], vsl,
                            pt[:, base + o0 - off: base + o1 - off],
                            start=grp not in started, stop=stop,
                        )
                        started.add(grp)
                        if stop and grp == "L":
                            ocl = ocp.tile([65, 256], BF16, name="ocl")
                            nc.vector.tensor_copy(ocl[:], av[:, 0:256])
                            nc.sync.dma_start(o[c, :, 0:256], ocl[:])

            budget = 0.0
            for p in range(p_lo, p_hi):
                need = {("B", (2 * p + 1) // 4), ("Vt", (2 * p + 1) // 4)}
                if p == p_lo:
                    need |= {("A", 2 * c), ("A", 2 * c + 1)}
                ensure(need)
                emit_scores(p)
                budget += per_pair
                while fillers and budget >= 1.0:
                    _, fn = fillers.pop(0)
                    fn()
                    budget -= 1.0
                if p >= p_lo + 1:
                    emit_av(p == p_lo + 1)
            emit_av(p_hi == p_lo + 1)
            if splitL:
                oc = ocp.tile([65, 256], BF16, name="oc")
                nc.vector.tensor_copy(oc[:], av[:, 256:512])
                nc.sync.dma_start(o[c, :, 256:512], oc[:])
            elif final:
                oc = ocp.tile([65, 512], BF16, name="oc")
                if c in parts:
                    nc.vector.tensor_add(oc[:], av[:], parts.pop(c)[:])
                else:
                    nc.vector.tensor_copy(oc[:], av[:])
                nc.sync.dma_start(o[c, :, :], oc[:])
            else:
                part = ocp.tile([65, 512], F32, tag="part", name="part")
                nc.vector.tensor_copy(part[:], av[:])
                parts[c] = part

        for _rep in range(repeats):
            # prologue: chunks 0,1 projected up front (B0 split for latency)
            for _, fn in (passB_units(0, split=2) + passA_units(0)
                          + vtrans_units(0) + passB_units(1, split=2)
                          + passA_units(1) + vtrans_units(1)):
                fn()

            f0 = (passB_units(2) + vtrans_units(2) + passA_units(2)
                  + passB_units(3) + vtrans_units(3) + passA_units(3))
            attn_seg(0, 0, 4, True, f0, per_pair=8.0)
            f1 = (f0 + passB_units(4) + vtrans_units(4) + passA_units(4)
                  + passB_units(5) + vtrans_units(5) + passA_units(5))
            attn_seg(1, 0, 8, True, f1, per_pair=4.0)
            f2 = (f1 + passA_units(6) + passA_units(7)
                  + passB_units(6) + vtrans_units(6))
            attn_seg(2, 0, 12, True, f2, per_pair=2.0)
            f3 = f2 + passB_units(7) + vtrans_units(7)
            attn_seg(3, 0, 16, True, f3, per_pair=1.0)
            for _, fn in f3:
                fn()

    nc.compile()
    return nc


def _get_nc():
    global _CACHED_NC
    if _CACHED_NC is None:
        _CACHED_NC = build_nc()
    return _CACHED_NC


def _host_inputs(x, wq, bq, wk, bk, wv, bv):
    bf = ml_dtypes.bfloat16
    wT = np.ascontiguousarray(
        np.concatenate([wv, wk, wq], axis=1).T
    ).astype(bf)  # [192, 768]
    xbf = np.ascontiguousarray(x).astype(bf)

    def mconst(live0):
        m = np.zeros((128, 592), np.float32)
        m[:, 0:128] = np.triu(np.ones((128, 128), np.float32))
        m[:, 128:512] = 1.0
        m[0:64, 512:576] = np.eye(64, dtype=np.float32)
        m[64:128, 576] = bq
        m[:, 578] = live0  # V' slot-0 ones column: 0 kills j=0's dead slot
        return np.ascontiguousarray(m.T).astype(bf)  # [592, 128]

    mT0, mT1 = mconst(0.0), mconst(1.0)
    in_maps = []
    for core in range(8):
        b, j = core // 2, core % 2
        if j == 0:
            xdev = np.concatenate(
                [np.zeros((128, DIN), bf), xbf[b][: SEQ - 128]], axis=0
            )
        else:
            xdev = xbf[b]
        in_maps.append({
            "x": np.ascontiguousarray(xdev),
            "wT": wT, "mT": mT0 if j == 0 else mT1,
        })
    return in_maps


def _assemble(results, bv):
    out = np.empty((4, SEQ, DOUT), np.float32)
    for core in range(8):
        b, j = core // 2, core % 2
        od = results[core]["o"]  # [NQC, 65, 512]
        for c in range(NQC):
            num = od[c, 0:64, :].astype(np.float64)
            den = od[c, 64, :].astype(np.float64)
            oc = (num / den).T.astype(np.float32) + bv  # [512, 64]
            for t in range(4):
                r0 = (8 * c + 2 * t + j) * 128
                out[b, r0:r0 + 128] = oc[t * 128:(t + 1) * 128]
    return out


def kernel(x, wq, bq, wk, bk, wv, bv):
    x = np.asarray(x, dtype=np.float32)
    args = [np.asarray(a, dtype=np.float32) for a in (wq, bq, wk, bk, wv, bv)]
    nc = _get_nc()
    in_maps = _host_inputs(x, *args)
    br = run_bass_kernel_spmd(nc, in_maps, core_ids=list(range(8)))
    return _assemble(br.results, args[5])


# revision 43
# speedup vs baseline: 1.4206x; 1.0107x over previous
"""Trainium2 Bass kernel: causal attention (QKV projection + causal softmax + AV).

Problem: x[4, 4096, 768] fp32, per-head projections to d=64, full causal
attention per batch, output [4, 4096, 64] fp32.

Sharding: 8 cores = 4 batches x 2 parity groups. Core (b, j) computes the
output rows of batch b whose 128-row block index i satisfies i % 2 == j.
One uniform SPMD program: for j=0 cores the host shifts x down by one
128-row block (prepending zeros), which makes the causal structure of both
parities identical in device coordinates (device q-blocks are always the odd
blocks 1,3,...,31; k-slot g holds true block g-1 for j=0 and g for j=1; the
dead slot 0 of j=0 is killed with a per-core -30000 exp bias).

Math shortcuts: bk is dropped (adds a per-row constant to scores ->
softmax-invariant); bv is added on the host after normalization
(sum of softmax weights is 1). Only bq is applied on device.

Device pipeline per core (all matmuls bf16, fp32 PSUM accumulation):
  proj: DMA-transposed x^T chunks (issued across 4 DGE queues) feed
     passB ([wv|wk] stationary -> V rows 0:64 / K rows 64:128 of PSUM;
     K copied to SBUF hi-half by Pool, V PE-transposed into V' = [V | 1])
     and passA (wq stationary -> Q^T in partitions 64:128 + bq via DVE).
  attn (per 512-col q chunk): per k-slot pair, two matmuls K^T_g.T @ Q^T
     into a [128, 1024] PSUM tile; exp on ACT (scale 1/8, per-core dead-slot
     bias, bf16 out); causal-diagonal mask mul on DVE; AV accumulates
     V'.T @ P^T into a [65, 512] PSUM tile whose row 64 is the softmax
     denominator. Scores run one pair ahead of AV and projection matmuls
     fill remaining PE slack so the PE never idles (keeps max p-state).
The unnormalized [65, 512] tiles go to DRAM; the host divides, adds bv,
and transposes.
"""

import numpy as np
import ml_dtypes
from contextlib import ExitStack

import concourse.bass as bass
import concourse.mybir as mybir
import concourse.tile as tile
from concourse import bacc
from concourse.bass_utils import run_bass_kernel_spmd

F32 = mybir.dt.float32
BF16 = mybir.dt.bfloat16

SEQ = 4096
DIN = 768
DOUT = 64
NCC = DIN // 128          # 6 contraction chunks
NSC = SEQ // 512          # 8 seq chunks (projection granularity)
NBLK = SEQ // 128         # 32 k-slots
NQC = 4                   # q chunks of 512 local columns (2048 own q rows)
SCALE = 1.0 / 8.0
EXPF = mybir.ActivationFunctionType.Exp

_CACHED_NC = None


def build_nc(repeats=1, pp=(8.0, 4.0, 2.0, 1.0)):
    nc = bacc.Bacc("TRN2", target_bir_lowering=False, debug=False)

    # x pre-transposed on host: [768, 4096] so every load is a plain
    # strided DMA (cheaper than XBAR transposes on the serial DMA device)
    xT = nc.dram_tensor("xT", [DIN, SEQ], BF16, kind="ExternalInput")
    wA = nc.dram_tensor("wA", [DIN, 192], BF16, kind="ExternalInput")  # [wv|wk|wq]
    mA = nc.dram_tensor("mA", [128, 592], BF16, kind="ExternalInput")
    o = nc.dram_tensor("o", [NQC, 65, 512], BF16, kind="ExternalOutput")

    with tile.TileContext(nc) as tc, ExitStack() as ctx:
        cpool = ctx.enter_context(tc.tile_pool(name="const", bufs=1))
        ptp = ctx.enter_context(tc.tile_pool(name="pt", bufs=4))
        ocp = ctx.enter_context(tc.tile_pool(name="oc", bufs=2))
        psproj = ctx.enter_context(tc.tile_pool(name="psproj", bufs=2, space="PSUM"))
        psst = ctx.enter_context(tc.tile_pool(name="psst", bufs=2, space="PSUM"))
        psav = ctx.enter_context(tc.tile_pool(name="psav", bufs=2, space="PSUM"))

        wsb = cpool.tile([128, NCC * 192], BF16)    # [wv|wk|wq] per cc chunk
        mc = cpool.tile([128, 592], BF16)           # mask | idn | bq | dead
        kvt = cpool.tile([128, NSC * 512], BF16)    # K^T rows 64:128, V^T rows 0:64
        xtf = cpool.tile([128, NSC * NCC * 512], BF16)  # x^T, whole sequence
        qt = cpool.tile([128, 16 * 128], BF16)      # Q^T own blocks, rows 64:128
        vs = cpool.tile([128, NBLK * 65], BF16)     # V' = [V | 1] per k-slot

        # ---- DMA issue plan: spread across 4 DGE queues so transfers
        # overlap and the first x^T chunk lands ~2us in.
        xT3 = xT.rearrange("(cc p) s -> p cc s", p=128)

        def trans(sc, eng, half=None, nsplit=2):
            """Plain strided load of x^T chunk sc (or a 1/nsplit piece)."""
            if half is None:
                c0, c1 = 0, 512
            else:
                w = 512 // nsplit
                c0, c1 = half * w, half * w + w
            s0 = sc * 512
            eng.dma_start(
                xtf[:, sc * NCC * 512:(sc + 1) * NCC * 512]
                .rearrange("p (cc s) -> p cc s", cc=NCC)[:, :, c0:c1],
                xT3[:, :, s0 + c0:s0 + c1],
            )

        # All loads are DmaTransposeAnt on the single sync (SP) queue: the
        # tile framework chains every DMA issue to the previous DMA's
        # completion EXCEPT consecutive same-kind same-queue DMAs, which
        # pipeline back-to-back on the (serial) DMA engine device.
        wv3 = wsb[:].rearrange("p (cc m) -> p cc m", cc=NCC)
        wA3 = wA.rearrange("(cc p) m -> p cc m", p=128)
        nc.sync.dma_start(wv3[:, 0:3, 0:128], wA3[:, 0:3, 0:128])
        trans(0, nc.sync, half=0)
        nc.sync.dma_start(wv3[:, 3:NCC, 0:128], wA3[:, 3:NCC, 0:128])
        trans(0, nc.sync, half=1)
        nc.sync.dma_start(wv3[:, :, 128:192], wA3[:, :, 128:192])
        nc.sync.dma_start(mc[:], mA[:, :])
        trans(1, nc.sync, half=0)
        trans(1, nc.sync, half=1)
        for _sc in range(2, NSC):
            trans(_sc, nc.sync)
        # f32 copy of bq (tensor_scalar needs f32 scalars)
        bqf = cpool.tile([128, 2], F32)
        nc.gpsimd.tensor_copy(bqf[:], mc[:, 576:578])
        # ones column of V'; slot 0 gets a per-core 0/1 (mc col 578) which
        # zeroes the j=0 dead slot's numerator AND denominator contribution
        nc.vector.memset(
            vs[:].rearrange("p (g e) -> p g e", g=NBLK)[:, :, 64:65], 1.0
        )
        nc.vector.tensor_copy(
            vs[:].rearrange("p (g e) -> p g e", g=NBLK)[:, 0:1, 64:65],
            mc[:, 578:579],
        )

        def xts(sc, cc):
            base = sc * NCC * 512 + cc * 512
            return xtf[:, base:base + 512]

        # ---- projection emitters; each returns a list of closures, one
        # per PE instruction (posts ride on the closure that needs them).
        proj_state = {}

        def passB_units(sc, split=False):
            """K^T rows 64:128 and V^T rows 0:64, [wv|wk] stationary."""
            units = []

            def mk(cc, c0, cols, start, stop):
                def f():
                    if cc == 0 and c0 == 0:
                        proj_state[("kp", sc)] = psproj.tile(
                            [128, 512], F32, tag="proj", name="kp")
                    kp = proj_state[("kp", sc)]
                    nc.tensor.matmul(
                        kp[:, c0:c0 + cols],
                        wsb[:, cc * 192:cc * 192 + 128],
                        xts(sc, cc)[:, c0:c0 + cols],
                        start=start, stop=stop,
                    )
                    if stop and c0 + cols == 512:
                        kp = proj_state.pop(("kp", sc))
                        # one copy: K rows 64:128 (no bias; bk is softmax-
                        # invariant) and V rows 0:64 land where each is used
                        nc.vector.tensor_copy(
                            kvt[:, sc * 512:(sc + 1) * 512], kp[:]
                        )
                return f

            if split:
                w = 512 // split
                for c0 in range(0, 512, w):
                    for cc in range(NCC):
                        units.append((("B", sc), mk(cc, c0, w, cc == 0, cc == NCC - 1)))
            else:
                for cc in range(NCC):
                    units.append((("B", sc), mk(cc, 0, 512, cc == 0, cc == NCC - 1)))
            return units

        def vtrans_units(sc):
            """V' blocks via PE transpose of vt, then one DVE copy to vs."""
            units = []

            def mk(t):
                def f():
                    if t == 0:
                        proj_state[("vp", sc)] = psproj.tile(
                            [128, 256], BF16, tag="proj", name="vp")
                    vp = proj_state[("vp", sc)]
                    nc.tensor.transpose(
                        vp[:, t * 64:(t + 1) * 64],
                        kvt[0:64, sc * 512 + t * 128: sc * 512 + (t + 1) * 128],
                        mc[0:64, 512:576],
                    )
                    if t == 3:
                        vp = proj_state.pop(("vp", sc))
                        nc.vector.tensor_copy(
                            vs[:].rearrange("p (g e) -> p g e", g=NBLK)[
                                :, sc * 4:(sc + 1) * 4, 0:64
                            ],
                            vp[:].rearrange("p (g e) -> p g e", g=4),
                        )
                return f

            for t in range(4):
                units.append((("Vt", sc), mk(t)))
            return units

        def passA_units(sc):
            """Q^T for own (odd) q-blocks of this chunk, into rows 64:128."""
            units = []

            def mk(cc):
                def f():
                    if cc == 0:
                        proj_state[("qp", sc)] = psproj.tile(
                            [128, 256], F32, tag="proj", name="qp")
                    qp = proj_state[("qp", sc)]
                    rhs = (
                        xts(sc, cc)
                        .rearrange("p (a b s) -> p a b s", a=2, b=2)[:, :, 1, :]
                    )
                    nc.tensor.matmul(
                        qp[64:128, :], wsb[:, cc * 192 + 128:cc * 192 + 192], rhs,
                        start=(cc == 0), stop=(cc == NCC - 1),
                    )
                    if cc == NCC - 1:
                        qp = proj_state.pop(("qp", sc))
                        nc.vector.tensor_scalar_add(
                            qt[64:128, sc * 256:(sc + 1) * 256],
                            qp[64:128, :], bqf[64:128, 0:1],
                        )
                return f

            for cc in range(NCC):
                units.append((("A", sc), mk(cc)))
            return units

        # ---- attention ----
        def slot_geom(c, g):
            s = g - (8 * c + 1)
            if s < 1:
                return 0, 512
            off = 128 * ((s + 1) // 2)
            return off, 512 - off

        parts = {}

        def attn_seg(c, p_lo, p_hi, final, fillers, per_pair):
            """Seg c pairs [p_lo, p_hi): scores one pair ahead of AV.

            fillers: ordered [(key, fn)] of projection units interleaved for
            PE occupancy; units a pair depends on are force-drained first.
            Non-final ranges park their partial AV in SBUF (parts).
            """
            npairs = 4 * c + 4
            av = psav.tile([65, 512], F32, tag="av", name="av")
            pending = []  # [(p, pt, geom)] awaiting AV

            def ensure(keys):
                while any(k in keys for k, _ in fillers):
                    _, fn = fillers.pop(0)
                    fn()

            def emit_scores(p):
                g0, g1 = 2 * p, 2 * p + 1
                off0, w0 = slot_geom(c, g0)
                off1, w1 = slot_geom(c, g1)
                st = psst.tile([128, 1024], F32, tag="st", name="st")
                if c == 0 and p < 2:
                    # the seg's second Q chunk lands late: columns [0:256]
                    # (first chunk's Q) can score and exp before the second
                    # passA completes; respects the compacted score layout
                    pt = ptp.tile([128, 1024], BF16, name="pt")
                    for cl, ch in ((0, 256), (256, 512)):
                        for g, base, off in ((g0, 0, off0), (g1, 512, off1)):
                            a = max(cl, off)
                            if a >= ch:
                                continue
                            nc.tensor.matmul(
                                st[:, base + a - off:base + ch - off],
                                kvt[64:128, g * 128:(g + 1) * 128],
                                qt[64:128, c * 512 + a:c * 512 + ch],
                                start=True, stop=True,
                            )
                            nc.scalar.activation(
                                pt[:, base + a - off:base + ch - off],
                                st[:, base + a - off:base + ch - off],
                                EXPF, bias=0.0, scale=SCALE)
                    if p >= npairs - 4:
                        nc.vector.tensor_mul(
                            pt[:, 512:512 + w1], pt[:, 512:512 + w1],
                            mc[:, 0:w1]
                        )
                    pending.append((p, pt, (off0, w0, off1, w1)))
                    return
                nc.tensor.matmul(
                    st[:, 0:w0], kvt[64:128, g0 * 128:(g0 + 1) * 128],
                    qt[64:128, c * 512 + off0: c * 512 + off0 + w0],
                    start=True, stop=True,
                )
                nc.tensor.matmul(
                    st[:, 512:512 + w1], kvt[64:128, g1 * 128:(g1 + 1) * 128],
                    qt[64:128, c * 512 + off1: c * 512 + off1 + w1],
                    start=True, stop=True,
                )
                pt = ptp.tile([128, 1024], BF16, name="pt")
                if w0 == 512:
                    nc.scalar.activation(pt[:, 0:512 + w1], st[:, 0:512 + w1],
                                         EXPF, bias=0.0, scale=SCALE)
                else:
                    # diagonal pairs have w0 == w1: one strided-AP exp covers
                    # both live regions [0:w0] and [512:512+w1]
                    sv = st[:].rearrange("p (j w) -> p j w", j=2)[:, :, 0:w0]
                    pv = pt[:].rearrange("p (j w) -> p j w", j=2)[:, :, 0:w0]
                    nc.scalar.activation(pv, sv, EXPF, bias=0.0, scale=SCALE)
                if p >= npairs - 4:
                    # odd member of the last four pairs is causal-diagonal
                    nc.vector.tensor_mul(
                        pt[:, 512:512 + w1], pt[:, 512:512 + w1], mc[:, 0:w1]
                    )
                pending.append((p, pt, (off0, w0, off1, w1)))

            splitL = final and c == 3 and p_hi == npairs
            started = set()

            def emit_av(first):
                p, pt, (off0, w0, off1, w1) = pending.pop(0)
                for g, off, base in ((2 * p, off0, 0),
                                     (2 * p + 1, off1, 512)):
                    vsl = vs[:, g * 65:(g + 1) * 65]
                    if not splitL:
                        nc.tensor.matmul(
                            av[:, off:512], vsl, pt[:, base:base + 512 - off],
                            start=first and base == 0,
                            stop=(p == p_hi - 1 and base == 512),
                        )
                        continue
                    # L=[0:256] closes at pair 13 (slot 27); R at pair 15
                    pieces = []
                    if off < 256:
                        pieces.append(("L", off, 256, g == 27))
                    pieces.append(("R", max(off, 256), 512, g == 31))
                    for grp, o0, o1, stop in pieces:
                        nc.tensor.matmul(
                            av[:, o0:o1], vsl,
                            pt[:, base + o0 - off: base + o1 - off],
                            start=grp not in started, stop=stop,
                        )
                        started.add(grp)
                        if stop and grp == "L":
                            ocl = ocp.tile([65, 256], BF16, name="ocl")
                            nc.vector.tensor_copy(ocl[:], av[:, 0:256])
                            nc.sync.dma_start(o[c, :, 0:256], ocl[:])

            budget = 0.0
            for p in range(p_lo, p_hi):
                need = {("B", (2 * p + 1) // 4), ("Vt", (2 * p + 1) // 4)}
                if p == p_lo:
                    need |= {("A", 2 * c), ("A", 2 * c + 1)}
                ensure(need)
                emit_scores(p)
                budget += per_pair
                while fillers and budget >= 1.0:
                    _, fn = fillers.pop(0)
                    fn()
                    budget -= 1.0
                if p >= p_lo + 1:
                    emit_av(p == p_lo + 1)
            emit_av(p_hi == p_lo + 1)
            if splitL:
                oc = ocp.tile([65, 256], BF16, name="oc")
                nc.vector.tensor_copy(oc[:], av[:, 256:512])
                nc.sync.dma_start(o[c, :, 256:512], oc[:])
            elif final:
                oc = ocp.tile([65, 512], BF16, name="oc")
                if c in parts:
                    nc.vector.tensor_add(oc[:], av[:], parts.pop(c)[:])
                else:
                    nc.vector.tensor_copy(oc[:], av[:])
                nc.sync.dma_start(o[c, :, :], oc[:])
            else:
                part = ocp.tile([65, 512], F32, tag="part", name="part")
                nc.vector.tensor_copy(part[:], av[:])
                parts[c] = part

        for _rep in range(repeats):
            # prologue: chunks 0,1 projected up front (B0 split for latency)
            for _, fn in (passB_units(0, split=2) + passA_units(0)
                          + vtrans_units(0) + passB_units(1, split=2)
                          + passA_units(1) + vtrans_units(1)):
                fn()

            f0 = (passB_units(2) + vtrans_units(2) + passA_units(2)
                  + passB_units(3) + vtrans_units(3) + passA_units(3))
            attn_seg(0, 0, 4, True, f0, per_pair=pp[0])
            f1 = (f0 + passB_units(4) + vtrans_units(4) + passA_units(4)
                  + passB_units(5) + vtrans_units(5) + passA_units(5))
            attn_seg(1, 0, 8, True, f1, per_pair=pp[1])
            f2 = (f1 + passA_units(6) + passA_units(7)
                  + passB_units(6) + vtrans_units(6))
            attn_seg(2, 0, 12, True, f2, per_pair=pp[2])
            f3 = f2 + passB_units(7) + vtrans_units(7)
            attn_seg(3, 0, 16, True, f3, per_pair=pp[3])
            for _, fn in f3:
                fn()

    nc.compile()
    return nc


def _get_nc():
    global _CACHED_NC
    if _CACHED_NC is None:
        _CACHED_NC = build_nc()
    return _CACHED_NC


def _host_inputs(x, wq, bq, wk, bk, wv, bv):
    bf = ml_dtypes.bfloat16
    wA = np.ascontiguousarray(
        np.concatenate([wv, wk, wq], axis=1)
    ).astype(bf)  # [768, 192]
    xbf = np.ascontiguousarray(x).astype(bf)

    def mconst(live0):
        m = np.zeros((128, 592), np.float32)
        m[:, 0:128] = np.triu(np.ones((128, 128), np.float32))
        m[:, 128:512] = 1.0
        m[0:64, 512:576] = np.eye(64, dtype=np.float32)
        m[64:128, 576] = bq
        m[:, 578] = live0  # V' slot-0 ones column: 0 kills j=0's dead slot
        return np.ascontiguousarray(m).astype(bf)  # [128, 592]

    mT0, mT1 = mconst(0.0), mconst(1.0)
    in_maps = []
    for core in range(8):
        b, j = core // 2, core % 2
        if j == 0:
            xdev = np.concatenate(
                [np.zeros((128, DIN), bf), xbf[b][: SEQ - 128]], axis=0
            )
        else:
            xdev = xbf[b]
        in_maps.append({
            "xT": np.ascontiguousarray(xdev.T),
            "wA": wA, "mA": mT0 if j == 0 else mT1,
        })
    return in_maps


def _assemble(results, bv):
    out = np.empty((4, SEQ, DOUT), np.float32)
    for core in range(8):
        b, j = core // 2, core % 2
        od = results[core]["o"]  # [NQC, 65, 512]
        for c in range(NQC):
            num = od[c, 0:64, :].astype(np.float64)
            den = od[c, 64, :].astype(np.float64)
            oc = (num / den).T.astype(np.float32) + bv  # [512, 64]
            for t in range(4):
                r0 = (8 * c + 2 * t + j) * 128
                out[b, r0:r0 + 128] = oc[t * 128:(t + 1) * 128]
    return out


def kernel(x, wq, bq, wk, bk, wv, bv):
    x = np.asarray(x, dtype=np.float32)
    args = [np.asarray(a, dtype=np.float32) for a in (wq, bq, wk, bk, wv, bv)]
    nc = _get_nc()
    in_maps = _host_inputs(x, *args)
    br = run_bass_kernel_spmd(nc, in_maps, core_ids=list(range(8)))
    return _assemble(br.results, args[5])


# revision 53
# speedup vs baseline: 1.4555x; 1.0245x over previous
"""Trainium2 Bass kernel: causal attention (QKV projection + causal softmax + AV).

Problem: x[4, 4096, 768] fp32, per-head projections to d=64, full causal
attention per batch, output [4, 4096, 64] fp32.

Sharding: 8 cores = 4 batches x 2 parity groups. Core (b, j) computes the
output rows of batch b whose 128-row block index i satisfies i % 2 == j.
One uniform SPMD program: for j=0 cores the host shifts x down by one
128-row block (prepending zeros), which makes the causal structure of both
parities identical in device coordinates (device q-blocks are always the odd
blocks 1,3,...,31; k-slot g holds true block g-1 for j=0 and g for j=1; the
dead slot 0 of j=0 is killed with a per-core -30000 exp bias).

Math shortcuts: bk is dropped (adds a per-row constant to scores ->
softmax-invariant); bv is added on the host after normalization
(sum of softmax weights is 1). Only bq is applied on device.

Device pipeline per core (all matmuls bf16, fp32 PSUM accumulation):
  proj: DMA-transposed x^T chunks (issued across 4 DGE queues) feed
     passB ([wv|wk] stationary -> V rows 0:64 / K rows 64:128 of PSUM;
     K copied to SBUF hi-half by Pool, V PE-transposed into V' = [V | 1])
     and passA (wq stationary -> Q^T in partitions 64:128 + bq via DVE).
  attn (per 512-col q chunk): per k-slot pair, two matmuls K^T_g.T @ Q^T
     into a [128, 1024] PSUM tile; exp on ACT (scale 1/8, per-core dead-slot
     bias, bf16 out); causal-diagonal mask mul on DVE; AV accumulates
     V'.T @ P^T into a [65, 512] PSUM tile whose row 64 is the softmax
     denominator. Scores run one pair ahead of AV and projection matmuls
     fill remaining PE slack so the PE never idles (keeps max p-state).
The unnormalized [65, 512] tiles go to DRAM; the host divides, adds bv,
and transposes.
"""

import numpy as np
import ml_dtypes
from contextlib import ExitStack

import concourse.bass as bass
import concourse.mybir as mybir
import concourse.tile as tile
from concourse import bacc
from concourse.bass_utils import run_bass_kernel_spmd

F32 = mybir.dt.float32
BF16 = mybir.dt.bfloat16

SEQ = 4096
DIN = 768
DOUT = 64
NCC = DIN // 128          # 6 contraction chunks
NSC = SEQ // 512          # 8 seq chunks (projection granularity)
NBLK = SEQ // 128         # 32 k-slots
NQC = 4                   # q chunks of 512 local columns (2048 own q rows)
SCALE = 1.0 / 8.0
EXPF = mybir.ActivationFunctionType.Exp

_CACHED_NC = None


def build_nc(repeats=1, pp=(8.0, 4.0, 1.5, 0.8), SOLO1=4, SOLO2=3):
    nc = bacc.Bacc("TRN2", target_bir_lowering=False, debug=False)

    # x pre-transposed on host: [768, 4096] so every load is a plain
    # strided DMA (cheaper than XBAR transposes on the serial DMA device)
    xT = nc.dram_tensor("xT", [DIN, SEQ], BF16, kind="ExternalInput")
    wA = nc.dram_tensor("wA", [DIN, 192], BF16, kind="ExternalInput")  # [wv|wk|wq]
    mA = nc.dram_tensor("mA", [128, 592], BF16, kind="ExternalInput")
    o = nc.dram_tensor("o", [NQC, 65, 512], BF16, kind="ExternalOutput")

    with tile.TileContext(nc) as tc, ExitStack() as ctx:
        cpool = ctx.enter_context(tc.tile_pool(name="const", bufs=1))
        ptp = ctx.enter_context(tc.tile_pool(name="pt", bufs=4))
        ocp = ctx.enter_context(tc.tile_pool(name="oc", bufs=2))
        psproj = ctx.enter_context(tc.tile_pool(name="psproj", bufs=2, space="PSUM"))
        psst = ctx.enter_context(tc.tile_pool(name="psst", bufs=2, space="PSUM"))
        psav = ctx.enter_context(tc.tile_pool(name="psav", bufs=2, space="PSUM"))

        wsb = cpool.tile([128, NCC * 192], BF16)    # [wv|wk|wq] per cc chunk
        mc = cpool.tile([128, 592], BF16)           # mask | idn | bq | dead
        kvt = cpool.tile([128, NSC * 512], BF16)    # K^T rows 64:128, V^T rows 0:64
        xtf = cpool.tile([128, NSC * NCC * 512], BF16)  # x^T, whole sequence
        qt = cpool.tile([128, 16 * 128], BF16)      # Q^T own blocks, rows 64:128
        vs = cpool.tile([128, NBLK * 65], BF16)     # V' = [V | 1] per k-slot

        # ---- DMA issue plan: spread across 4 DGE queues so transfers
        # overlap and the first x^T chunk lands ~2us in.
        xT3 = xT.rearrange("(cc p) s -> p cc s", p=128)

        def trans(sc, eng, half=None, nsplit=2):
            """Plain strided load of x^T chunk sc (or a 1/nsplit piece)."""
            if half is None:
                c0, c1 = 0, 512
            else:
                w = 512 // nsplit
                c0, c1 = half * w, half * w + w
            s0 = sc * 512
            eng.dma_start(
                xtf[:, sc * NCC * 512:(sc + 1) * NCC * 512]
                .rearrange("p (cc s) -> p cc s", cc=NCC)[:, :, c0:c1],
                xT3[:, :, s0 + c0:s0 + c1],
            )

        # All loads are DmaTransposeAnt on the single sync (SP) queue: the
        # tile framework chains every DMA issue to the previous DMA's
        # completion EXCEPT consecutive same-kind same-queue DMAs, which
        # pipeline back-to-back on the (serial) DMA engine device.
        wv3 = wsb[:].rearrange("p (cc m) -> p cc m", cc=NCC)
        wA3 = wA.rearrange("(cc p) m -> p cc m", p=128)
        nc.sync.dma_start(wv3[:, 0:3, 0:128], wA3[:, 0:3, 0:128])
        trans(0, nc.sync, half=0)
        nc.sync.dma_start(wv3[:, 3:NCC, 0:128], wA3[:, 3:NCC, 0:128])
        trans(0, nc.sync, half=1)
        nc.sync.dma_start(wv3[:, :, 128:192], wA3[:, :, 128:192])
        nc.sync.dma_start(mc[:], mA[:, :])
        trans(1, nc.sync, half=0)
        trans(1, nc.sync, half=1)
        for _sc in range(2, NSC):
            trans(_sc, nc.sync)
        # f32 copy of bq (tensor_scalar needs f32 scalars)
        bqf = cpool.tile([128, 2], F32)
        nc.gpsimd.tensor_copy(bqf[:], mc[:, 576:578])
        # ones column of V'; slot 0 gets a per-core 0/1 (mc col 578) which
        # zeroes the j=0 dead slot's numerator AND denominator contribution
        nc.vector.memset(
            vs[:].rearrange("p (g e) -> p g e", g=NBLK)[:, :, 64:65], 1.0
        )
        nc.vector.tensor_copy(
            vs[:].rearrange("p (g e) -> p g e", g=NBLK)[:, 0:1, 64:65],
            mc[:, 578:579],
        )

        def xts(sc, cc):
            base = sc * NCC * 512 + cc * 512
            return xtf[:, base:base + 512]

        # ---- projection emitters; each returns a list of closures, one
        # per PE instruction (posts ride on the closure that needs them).
        proj_state = {}

        def passB_units(sc, split=False):
            """K^T rows 64:128 and V^T rows 0:64, [wv|wk] stationary."""
            units = []

            def mk(cc, c0, cols, start, stop):
                def f():
                    if cc == 0 and c0 == 0:
                        proj_state[("kp", sc)] = psproj.tile(
                            [128, 512], F32, tag="proj", name="kp")
                    kp = proj_state[("kp", sc)]
                    nc.tensor.matmul(
                        kp[:, c0:c0 + cols],
                        wsb[:, cc * 192:cc * 192 + 128],
                        xts(sc, cc)[:, c0:c0 + cols],
                        start=start, stop=stop,
                    )
                    if stop and c0 + cols == 512:
                        kp = proj_state.pop(("kp", sc))
                        # one copy: K rows 64:128 (no bias; bk is softmax-
                        # invariant) and V rows 0:64 land where each is used
                        nc.vector.tensor_copy(
                            kvt[:, sc * 512:(sc + 1) * 512], kp[:]
                        )
                return f

            if split:
                w = 512 // split
                for c0 in range(0, 512, w):
                    for cc in range(NCC):
                        units.append((("B", sc), mk(cc, c0, w, cc == 0, cc == NCC - 1)))
            else:
                for cc in range(NCC):
                    units.append((("B", sc), mk(cc, 0, 512, cc == 0, cc == NCC - 1)))
            return units

        def vtrans_units(sc):
            """V' blocks via PE transpose of vt, then one DVE copy to vs."""
            units = []

            def mk(t):
                def f():
                    if t == 0:
                        proj_state[("vp", sc)] = psproj.tile(
                            [128, 256], BF16, tag="proj", name="vp")
                    vp = proj_state[("vp", sc)]
                    nc.tensor.transpose(
                        vp[:, t * 64:(t + 1) * 64],
                        kvt[0:64, sc * 512 + t * 128: sc * 512 + (t + 1) * 128],
                        mc[0:64, 512:576],
                    )
                    if t == 3:
                        vp = proj_state.pop(("vp", sc))
                        nc.vector.tensor_copy(
                            vs[:].rearrange("p (g e) -> p g e", g=NBLK)[
                                :, sc * 4:(sc + 1) * 4, 0:64
                            ],
                            vp[:].rearrange("p (g e) -> p g e", g=4),
                        )
                return f

            for t in range(4):
                units.append((("Vt", sc), mk(t)))
            return units

        def passA_units(sc):
            """Q^T for own (odd) q-blocks of this chunk, into rows 64:128."""
            units = []

            def mk(cc):
                def f():
                    if cc == 0:
                        proj_state[("qp", sc)] = psproj.tile(
                            [128, 256], F32, tag="proj", name="qp")
                    qp = proj_state[("qp", sc)]
                    rhs = (
                        xts(sc, cc)
                        .rearrange("p (a b s) -> p a b s", a=2, b=2)[:, :, 1, :]
                    )
                    nc.tensor.matmul(
                        qp[64:128, :], wsb[:, cc * 192 + 128:cc * 192 + 192], rhs,
                        start=(cc == 0), stop=(cc == NCC - 1),
                    )
                    if cc == NCC - 1:
                        qp = proj_state.pop(("qp", sc))
                        nc.vector.tensor_scalar_add(
                            qt[64:128, sc * 256:(sc + 1) * 256],
                            qp[64:128, :], bqf[64:128, 0:1],
                        )
                return f

            for cc in range(NCC):
                units.append((("A", sc), mk(cc)))
            return units

        # ---- attention ----
        def slot_geom(c, g):
            s = g - (8 * c + 1)
            if s < 1:
                return 0, 512
            off = 128 * ((s + 1) // 2)
            return off, 512 - off

        parts = {}

        def attn_seg(c, p_lo, p_hi, final, fillers, per_pair):
            for _ in attn_seg_gen(c, p_lo, p_hi, final, fillers, per_pair):
                pass

        def attn_seg_gen(c, p_lo, p_hi, final, fillers, per_pair):
            """Seg c pairs [p_lo, p_hi): scores one pair ahead of AV.

            fillers: ordered [(key, fn)] of projection units interleaved for
            PE occupancy; units a pair depends on are force-drained first.
            Non-final ranges park their partial AV in SBUF (parts).
            """
            npairs = 4 * c + 4
            av = psav.tile([65, 512], F32, tag="av", name="av")
            pending = []  # [(p, pt, geom)] awaiting AV

            def ensure(keys):
                while any(k in keys for k, _ in fillers):
                    _, fn = fillers.pop(0)
                    fn()

            def emit_scores(p):
                g0, g1 = 2 * p, 2 * p + 1
                off0, w0 = slot_geom(c, g0)
                off1, w1 = slot_geom(c, g1)
                st = psst.tile([128, 1024], F32, tag="st", name="st")
                if c == 0 and p < 2:
                    # the seg's second Q chunk lands late: columns [0:256]
                    # (first chunk's Q) can score and exp before the second
                    # passA completes; respects the compacted score layout
                    pt = ptp.tile([128, 1024], BF16, name="pt")
                    for cl, ch in ((0, 256), (256, 512)):
                        for g, base, off in ((g0, 0, off0), (g1, 512, off1)):
                            a = max(cl, off)
                            if a >= ch:
                                continue
                            nc.tensor.matmul(
                                st[:, base + a - off:base + ch - off],
                                kvt[64:128, g * 128:(g + 1) * 128],
                                qt[64:128, c * 512 + a:c * 512 + ch],
                                start=True, stop=True,
                            )
                            nc.scalar.activation(
                                pt[:, base + a - off:base + ch - off],
                                st[:, base + a - off:base + ch - off],
                                EXPF, bias=0.0, scale=SCALE)
                    if p >= npairs - 4:
                        nc.vector.tensor_mul(
                            pt[:, 512:512 + w1], pt[:, 512:512 + w1],
                            mc[:, 0:w1]
                        )
                    pending.append((p, pt, (off0, w0, off1, w1)))
                    return
                nc.tensor.matmul(
                    st[:, 0:w0], kvt[64:128, g0 * 128:(g0 + 1) * 128],
                    qt[64:128, c * 512 + off0: c * 512 + off0 + w0],
                    start=True, stop=True,
                )
                nc.tensor.matmul(
                    st[:, 512:512 + w1], kvt[64:128, g1 * 128:(g1 + 1) * 128],
                    qt[64:128, c * 512 + off1: c * 512 + off1 + w1],
                    start=True, stop=True,
                )
                pt = ptp.tile([128, 1024], BF16, name="pt")
                if w0 == 512:
                    nc.scalar.activation(pt[:, 0:512 + w1], st[:, 0:512 + w1],
                                         EXPF, bias=0.0, scale=SCALE)
                else:
                    # diagonal pairs have w0 == w1: one strided-AP exp covers
                    # both live regions [0:w0] and [512:512+w1]
                    sv = st[:].rearrange("p (j w) -> p j w", j=2)[:, :, 0:w0]
                    pv = pt[:].rearrange("p (j w) -> p j w", j=2)[:, :, 0:w0]
                    nc.scalar.activation(pv, sv, EXPF, bias=0.0, scale=SCALE)
                if p >= npairs - 4:
                    # odd member of the last four pairs is causal-diagonal
                    nc.vector.tensor_mul(
                        pt[:, 512:512 + w1], pt[:, 512:512 + w1], mc[:, 0:w1]
                    )
                pending.append((p, pt, (off0, w0, off1, w1)))

            splitL = final and c == 3 and p_hi == npairs
            started = set()

            def emit_av(first):
                p, pt, (off0, w0, off1, w1) = pending.pop(0)
                for g, off, base in ((2 * p, off0, 0),
                                     (2 * p + 1, off1, 512)):
                    vsl = vs[:, g * 65:(g + 1) * 65]
                    if not splitL:
                        nc.tensor.matmul(
                            av[:, off:512], vsl, pt[:, base:base + 512 - off],
                            start=first and base == 0,
                            stop=(p == p_hi - 1 and base == 512),
                        )
                        continue
                    # L=[0:256] closes at pair 13 (slot 27); R at pair 15
                    pieces = []
                    if off < 256:
                        pieces.append(("L", off, 256, g == 27))
                    pieces.append(("R", max(off, 256), 512, g == 31))
                    for grp, o0, o1, stop in pieces:
                        nc.tensor.matmul(
                            av[:, o0:o1], vsl,
                            pt[:, base + o0 - off: base + o1 - off],
                            start=grp not in started, stop=stop,
                        )
                        started.add(grp)
                        if stop and grp == "L":
                            ocl = ocp.tile([65, 256], BF16, name="ocl")
                            nc.vector.tensor_copy(ocl[:], av[:, 0:256])
                            nc.sync.dma_start(o[c, :, 0:256], ocl[:])

            budget = 0.0
            for p in range(p_lo, p_hi):
                need = {("B", (2 * p + 1) // 4), ("Vt", (2 * p + 1) // 4)}
                if p == p_lo:
                    need |= {("A", 2 * c), ("A", 2 * c + 1)}
                ensure(need)
                emit_scores(p)
                budget += per_pair
                while fillers and budget >= 1.0:
                    _, fn = fillers.pop(0)
                    fn()
                    budget -= 1.0
                if p >= p_lo + 1:
                    emit_av(p == p_lo + 1)
                yield
            emit_av(p_hi == p_lo + 1)
            if splitL:
                oc = ocp.tile([65, 256], BF16, name="oc")
                nc.vector.tensor_copy(oc[:], av[:, 256:512])
                nc.sync.dma_start(o[c, :, 256:512], oc[:])
            elif final:
                oc = ocp.tile([65, 512], BF16, name="oc")
                if c in parts:
                    nc.vector.tensor_add(oc[:], av[:], parts.pop(c)[:])
                else:
                    nc.vector.tensor_copy(oc[:], av[:])
                nc.sync.dma_start(o[c, :, :], oc[:])
            else:
                part = ocp.tile([65, 512], F32, tag="part", name="part")
                nc.vector.tensor_copy(part[:], av[:])
                parts[c] = part

        for _rep in range(repeats):
            # prologue: chunks 0,1 projected up front (B0 split for latency)
            for _, fn in (passB_units(0, split=2) + passA_units(0)
                          + vtrans_units(0) + passB_units(1, split=2)
                          + passA_units(1) + vtrans_units(1)):
                fn()

            f0 = (passB_units(2) + vtrans_units(2) + passA_units(2)
                  + passB_units(3) + vtrans_units(3) + passA_units(3))
            attn_seg(0, 0, 4, True, f0, per_pair=pp[0])
            f1 = f0
            f1x = (f1 + passB_units(4) + vtrans_units(4) + passA_units(4)
                   + passB_units(5) + vtrans_units(5) + passA_units(5))
            attn_seg(1, 0, 8, True, f1x, pp[1])
            f23 = (f1x + passA_units(6) + passA_units(7)
                   + passB_units(6) + vtrans_units(6)
                   + passB_units(7) + vtrans_units(7))
            g2 = attn_seg_gen(2, 0, 12, True, f23, pp[2])
            g3 = attn_seg_gen(3, 0, 16, True, f23, pp[3])

            def step(g):
                try:
                    next(g)
                    return True
                except StopIteration:
                    return False

            for _ in range(SOLO2):
                step(g2)
            # alternate the two segs: doubles the effective score-buffer
            # rotation depth, hiding the exp->scores semaphore latency
            a2 = a3 = True
            while a2 or a3:
                if a3:
                    a3 = step(g3)
                if a2:
                    a2 = step(g2)
            for _, fn in f23:
                fn()

    nc.compile()
    return nc


def _get_nc():
    global _CACHED_NC
    if _CACHED_NC is None:
        _CACHED_NC = build_nc()
    return _CACHED_NC


def _host_inputs(x, wq, bq, wk, bk, wv, bv):
    bf = ml_dtypes.bfloat16
    wA = np.ascontiguousarray(
        np.concatenate([wv, wk, wq], axis=1)
    ).astype(bf)  # [768, 192]
    xbf = np.ascontiguousarray(x).astype(bf)

    def mconst(live0):
        m = np.zeros((128, 592), np.float32)
        m[:, 0:128] = np.triu(np.ones((128, 128), np.float32))
        m[:, 128:512] = 1.0
        m[0:64, 512:576] = np.eye(64, dtype=np.float32)
        m[64:128, 576] = bq
        m[:, 578] = live0  # V' slot-0 ones column: 0 kills j=0's dead slot
        return np.ascontiguousarray(m).astype(bf)  # [128, 592]

    mT0, mT1 = mconst(0.0), mconst(1.0)
    in_maps = []
    for core in range(8):
        b, j = core // 2, core % 2
        if j == 0:
            xdev = np.concatenate(
                [np.zeros((128, DIN), bf), xbf[b][: SEQ - 128]], axis=0
            )
        else:
            xdev = xbf[b]
        in_maps.append({
            "xT": np.ascontiguousarray(xdev.T),
            "wA": wA, "mA": mT0 if j == 0 else mT1,
        })
    return in_maps


def _assemble(results, bv):
    out = np.empty((4, SEQ, DOUT), np.float32)
    for core in range(8):
        b, j = core // 2, core % 2
        od = results[core]["o"]  # [NQC, 65, 512]
        for c in range(NQC):
            num = od[c, 0:64, :].astype(np.float64)
            den = od[c, 64, :].astype(np.float64)
            oc = (num / den).T.astype(np.float32) + bv  # [512, 64]
            for t in range(4):
                r0 = (8 * c + 2 * t + j) * 128
                out[b, r0:r0 + 128] = oc[t * 128:(t + 1) * 128]
    return out


def kernel(x, wq, bq, wk, bk, wv, bv):
    x = np.asarray(x, dtype=np.float32)
    args = [np.asarray(a, dtype=np.float32) for a in (wq, bq, wk, bk, wv, bv)]
    nc = _get_nc()
    in_maps = _host_inputs(x, *args)
    br = run_bass_kernel_spmd(nc, in_maps, core_ids=list(range(8)))
    return _assemble(br.results, args[5])
